# revision 1
# baseline (speedup 1.0000x reference)
"""Spatial-reduction attention (PVT-style) on 8 TRN2 NeuronCores.

Strategy: pure data-parallel over batch (B=8 -> 1 batch element per core,
zero collectives). Per core, everything is computed in "feature-major"
(transposed) layout so that the attention-weight matrix E^T = exp(S^T)
lands with the context dim m on partitions -- exactly what the PV matmul
needs as its moving operand, so the big attention tensor is never
transposed on chip.

Key tricks:
  - conv(stride 2, 2x2) == patch-merge matmul; patches are gathered
    host-side, only for the m positions with mask!=0 (mask compression,
    1024 -> M_pad ~ 640), since masked context positions contribute
    nothing to the attention output.
  - mask + softmax denominator are folded into the PV matmul: the
    stationary operand V'' has 65 columns per head (64 = mask*V, 1 = mask),
    so row 64 of the PV output is the softmax denominator. No masking work
    on the big [m, n] tensor at all, and no max-subtraction (scores are
    O(1)-scaled by construction, exp cannot overflow).
  - layernorm's ln_w/ln_b are folded into Wkv host-side; on-chip LN is a
    pure standardize using ones-matmul column stats + partition broadcast.
  - all matmuls run in bf16 (full PE rate; rel err ~7e-3, gate is 2e-2).
  - output is produced transposed ([512, 4096] per core) and untransposed
    on the host.

Measured: 326 us HW exec (neuron-profile exec_time_ns), rel err 7.5e-3 vs
the fp32 reference (gate 2e-2). Breakdown at this point: ~65 us prologue
(patch/LN/KV chain), ~215 us attention steady-state (ACT exp is the
pacer at ~75% duty; PE warm at 2.4 GHz), ~30 us tail + Tile drain.
"""

import math
import numpy as np

N_SEQ = 4096
DIM = 512
HEADS = 8
DH = 64
INNER = HEADS * DH
SR = 2
SCALE = DH ** -0.5
LN_EPS = 1e-5
B = 8
NCHUNK = 512          # n-tile size of the main loop
EH = DH + 1           # 65: V'' columns per head (64 V + 1 mask/denominator)


def _ensure_path():
    try:
        import concourse.bass  # noqa: F401
    except ImportError:
        import sys
        for p in ("/opt/trn_rl_repo", "/root/.axon_site/_ro/trn_rl_repo"):
            if p not in sys.path:
                sys.path.append(p)


def _m_pieces(m_pad):
    """Split [0, m_pad) into 128-aligned pieces of at most 512, so each
    piece covers whole m-chunks (lets attention start on piece 1 early)."""
    if m_pad <= 512:
        return [(0, m_pad)]
    nmc = m_pad // 128
    a = min(3, nmc - 1)
    return [(0, a * 128), (a * 128, m_pad)]


def _build(m_pad):
    _ensure_path()
    import concourse.bass as bass  # noqa: F401
    import concourse.mybir as mybir
    import concourse.tile as tile
    from concourse import bacc

    f32 = mybir.dt.float32
    bf16 = mybir.dt.bfloat16
    FT = mybir.ActivationFunctionType
    OP = mybir.AluOpType

    nmc = m_pad // 128
    pieces = _m_pieces(m_pad)
    n_nc = N_SEQ // NCHUNK

    nc = bacc.Bacc()

    xt_e = nc.declare_dram_parameter("xt", [DIM, N_SEQ], bf16, isOutput=False)
    xp_e = nc.declare_dram_parameter("xp", [4 * DIM, m_pad], bf16, isOutput=False)
    w2_e = nc.declare_dram_parameter("w2", [4 * DIM, DIM], bf16, isOutput=False)
    wq_e = nc.declare_dram_parameter("wq", [128, 4, DIM], bf16, isOutput=False)
    wk_e = nc.declare_dram_parameter("wk", [128, 4, DIM], bf16, isOutput=False)
    wv_e = nc.declare_dram_parameter("wv", [128, 4, DIM], bf16, isOutput=False)
    wp_e = nc.declare_dram_parameter("wp", [128, 4, DIM], bf16, isOutput=False)
    convb_e = nc.declare_dram_parameter("convb", [128, 4], f32, isOutput=False)
    bk_e = nc.declare_dram_parameter("bk", [128, 4], f32, isOutput=False)
    bv_e = nc.declare_dram_parameter("bv", [DIM], f32, isOutput=False)
    bp_e = nc.declare_dram_parameter("bp", [128, 4], f32, isOutput=False)
    maskc_e = nc.declare_dram_parameter("maskc", [128, nmc], f32, isOutput=False)
    out_e = nc.declare_dram_parameter("out", [DIM, N_SEQ], f32, isOutput=True)

    def r(ap):
        return ap

    from contextlib import ExitStack

    with tile.TileContext(nc) as tc:
        with ExitStack() as stk:
            def pool(name, bufs, space="SBUF"):
                return stk.enter_context(
                    tc.tile_pool(name=name, bufs=bufs, space=space))

            wpool = pool("wts", 1)
            cpool = pool("consts", 1)
            xpcp = pool("stream", 1)
            w2p = pool("w2s", 1)
            ctxp = pool("ctx", 1)
            sqp = pool("sqs", 2)
            kvp = pool("kv", 1)
            xtp = pool("xtq", 2)
            qp = pool("qq", 2)
            ep = pool("ee", 2)
            opool = pool("oo", 2)
            yp = pool("yy", 3)
            smp = pool("small", 1)
            r1p = pool("r1p", 2)
            bcp = pool("bc", 1)
            rbp = pool("rbp", 2)
            spool = pool("ps_s", 1, space="PSUM")
            pvp = pool("ps_pv", 1, space="PSUM")
            qpp = pool("ps_q", 1, space="PSUM")
            fpp = pool("ps_f", 1, space="PSUM")
            # ---- constants / weights (resident) ----
            wq_sb = wpool.tile([128, 4, DIM], bf16, tag="wq")
            nc.gpsimd.dma_start(out=wq_sb[:], in_=wq_e.ap())
            wk_sb = wpool.tile([128, 4, DIM], bf16, tag="wk")
            nc.gpsimd.dma_start(out=wk_sb[:], in_=wk_e.ap())
            wv_sb = wpool.tile([128, 4, DIM], bf16, tag="wv")
            nc.gpsimd.dma_start(out=wv_sb[:], in_=wv_e.ap())
            wp_sb = wpool.tile([128, 4, DIM], bf16, tag="wp")
            nc.gpsimd.dma_start(out=wp_sb[:], in_=wp_e.ap())
            convb_sb = cpool.tile([128, 4], f32, tag="convb")
            nc.gpsimd.dma_start(out=convb_sb[:], in_=convb_e.ap())
            bk_sb = cpool.tile([128, 4], f32, tag="bk")
            nc.gpsimd.dma_start(out=bk_sb[:], in_=bk_e.ap())
            bp_sb = cpool.tile([128, 4], f32, tag="bp")
            nc.gpsimd.dma_start(out=bp_sb[:], in_=bp_e.ap())
            maskc_sb = cpool.tile([128, nmc], f32, tag="maskc")
            nc.gpsimd.dma_start(out=maskc_sb[:], in_=maskc_e.ap())
            bv_bc = cpool.tile([128, DIM], f32, tag="bvbc")
            nc.gpsimd.dma_start(out=bv_bc[:], in_=bv_e.ap().partition_broadcast(128))
            ones_sb = cpool.tile([128, 1], bf16, tag="ones")
            nc.vector.memset(ones_sb[:], 1.0)
            ones8_sb = cpool.tile([128, 8], f32, tag="ones8")
            nc.vector.memset(ones8_sb[:], 1.0)
            eps_sb = cpool.tile([1, 1], f32, tag="eps")
            nc.vector.memset(eps_sb[:], LN_EPS)

            # ---- phase 1: patch-merge ctx^T (feature-major) + layernorm ----
            ctx_raw = ctxp.tile([128, 4, m_pad], bf16, tag="craw")
            ctxn = ctxp.tile([128, 4, m_pad], bf16, tag="cn")

            xp_r = xp_e.ap().rearrange("(kc p) m -> kc p m", p=128)   # [16,128,m]
            w2_r = w2_e.ap().rearrange("(kc p) co -> kc p co", p=128)  # [16,128,512]

            xt_r0 = xt_e.ap().rearrange("(cc p) n -> p cc n", p=128)
            xt_pre = xtp.tile([128, 4, NCHUNK], bf16, tag="xt")
            nc.sync.dma_start(out=xt_pre[:], in_=xt_r0[:, :, 0:NCHUNK])

            xp_t = []
            w2_t = []
            for kc in range(16):
                xpk = xpcp.tile([128, m_pad], bf16, tag=f"xp{kc}")
                nc.sync.dma_start(out=xpk[:], in_=xp_r[kc, :, :])
                xp_t.append(xpk)
                w2k = w2p.tile([128, DIM], bf16, tag=f"w2{kc}")
                nc.sync.dma_start(out=w2k[:], in_=w2_r[kc, :, :])
                w2_t.append(w2k)

            k_sb = kvp.tile([128, 4, m_pad], bf16, tag="k")
            k_sw = kvp.tile([128, 4, m_pad], bf16, tag="ksw")
            v2_sb = kvp.tile([128, nmc, HEADS * EH], bf16, tag="v2")
            bv3 = bv_bc[:].rearrange("p (h d) -> p h d", d=DH)

            for (p0, p1) in pieces:
                pw = p1 - p0
                # c_out-chunk accumulators live in the (idle) scores banks
                psA = spool.tile([128, 3, NCHUNK], f32, tag="sA")
                psB = spool.tile([128, 2, NCHUNK], f32, tag="sB")
                for kc in range(16):
                    for cco in range(4):
                        dst = psA[:, cco, :pw] if cco < 3 else psB[:, cco - 3, :pw]
                        nc.tensor.matmul(
                            dst,
                            lhsT=r(w2_t[kc][:, cco * 128:(cco + 1) * 128]),
                            rhs=r(xp_t[kc][:, p0:p1]),
                            start=(kc == 0), stop=(kc == 15),
                        )
                for cco in range(4):
                    src = psA[:, cco, :pw] if cco < 3 else psB[:, cco - 3, :pw]
                    nc.scalar.activation(
                        out=ctx_raw[:, cco, p0:p1], in_=src,
                        func=FT.Identity, bias=convb_sb[:, cco:cco + 1],
                    )
                # column stats via ones-matmul (sum over the c partition dim)
                mu_ps = pvp.tile([EH, NCHUNK], f32, tag="pv")
                ss_ps = fpp.tile([128, NCHUNK], f32, tag="fin")
                for cc in range(4):
                    sq_s = sqp.tile([128, NCHUNK], bf16, tag="sqs")
                    nc.scalar.activation(
                        out=sq_s[:, :pw], in_=ctx_raw[:, cc, p0:p1],
                        func=FT.Square,
                    )
                    nc.tensor.matmul(
                        mu_ps[0:1, :pw], lhsT=r(ones_sb[:]),
                        rhs=r(ctx_raw[:, cc, p0:p1]),
                        start=(cc == 0), stop=(cc == 3),
                    )
                    nc.tensor.matmul(
                        ss_ps[0:1, :pw], lhsT=r(ones_sb[:]),
                        rhs=r(sq_s[:, :pw]),
                        start=(cc == 0), stop=(cc == 3),
                    )
                m1n = smp.tile([1, NCHUNK], f32, tag="m1n")
                nc.vector.tensor_scalar(
                    out=m1n[:, :pw], in0=mu_ps[0:1, :pw],
                    scalar1=-1.0 / DIM, scalar2=None, op0=OP.mult,
                )
                v1 = smp.tile([1, NCHUNK], f32, tag="v1")
                nc.vector.tensor_scalar(
                    out=v1[:, :pw], in0=ss_ps[0:1, :pw],
                    scalar1=1.0 / DIM, scalar2=None, op0=OP.mult,
                )
                m2 = smp.tile([1, NCHUNK], f32, tag="m2")
                nc.vector.tensor_tensor(
                    out=m2[:, :pw], in0=m1n[:, :pw], in1=m1n[:, :pw], op=OP.mult
                )
                var = smp.tile([1, NCHUNK], f32, tag="var")
                nc.vector.tensor_tensor(
                    out=var[:, :pw], in0=v1[:, :pw], in1=m2[:, :pw], op=OP.subtract
                )
                std = smp.tile([1, NCHUNK], f32, tag="std")
                nc.scalar.activation(
                    out=std[:, :pw], in_=var[:, :pw], func=FT.Sqrt,
                    bias=eps_sb[:],
                )
                rstd = smp.tile([1, NCHUNK], f32, tag="rstd")
                nc.vector.reciprocal(out=rstd[:, :pw], in_=std[:, :pw])
                tsh = smp.tile([1, NCHUNK], f32, tag="tsh")
                nc.vector.tensor_tensor(
                    out=tsh[:, :pw], in0=m1n[:, :pw], in1=rstd[:, :pw], op=OP.mult
                )
                r_bc = bcp.tile([128, NCHUNK], f32, tag="rbc")
                nc.gpsimd.partition_broadcast(out_ap=r_bc[:, :pw], in_ap=rstd[:, :pw])
                t_bc = bcp.tile([128, NCHUNK], f32, tag="tbc")
                nc.gpsimd.partition_broadcast(out_ap=t_bc[:, :pw], in_ap=tsh[:, :pw])
                for cc in range(4):
                    nc.vector.tensor_tensor(
                        out=ctxn[:, cc, p0:p1], in0=ctx_raw[:, cc, p0:p1],
                        in1=r_bc[:, :pw], op=OP.mult,
                    )
                    nc.vector.tensor_tensor(
                        out=ctxn[:, cc, p0:p1], in0=ctxn[:, cc, p0:p1],
                        in1=t_bc[:, :pw], op=OP.add,
                    )
                # K^T (feature-major) for this piece
                for kc in range(4):
                    ps = qpp.tile([128, NCHUNK], f32, tag="q")
                    for cc in range(4):
                        nc.tensor.matmul(
                            ps[:, :pw],
                            lhsT=r(wk_sb[:, cc, kc * 128:(kc + 1) * 128]),
                            rhs=r(ctxn[:, cc, p0:p1]),
                            start=(cc == 0), stop=(cc == 3),
                        )
                    nc.scalar.activation(
                        out=k_sb[:, kc, p0:p1], in_=ps[:, :pw],
                        func=FT.Identity, bias=bk_sb[:, kc:kc + 1],
                    )
                nc.vector.tensor_copy(
                    out=k_sw[0:64, :, p0:p1], in_=k_sb[64:128, :, p0:p1])
                nc.vector.tensor_copy(
                    out=k_sw[64:128, :, p0:p1], in_=k_sb[0:64, :, p0:p1])
                # V'' (token-major) for this piece's m-chunks
                for mc in range(p0 // 128, p1 // 128):
                    ps = fpp.tile([128, NCHUNK], f32, tag="fin")
                    for cc in range(4):
                        nc.tensor.matmul(
                            ps[:],
                            lhsT=r(ctxn[:, cc, mc * 128:(mc + 1) * 128]),
                            rhs=r(wv_sb[:, cc, :]),
                            start=(cc == 0), stop=(cc == 3),
                        )
                    v3 = v2_sb[:, mc, :].rearrange("p (h e) -> p h e", e=EH)
                    nc.vector.tensor_tensor(
                        out=v3[:, :, 0:DH],
                        in0=ps[:].rearrange("p (h d) -> p h d", d=DH),
                        in1=bv3, op=OP.add,
                    )
                    nc.vector.tensor_scalar(
                        out=v3[:, :, 0:DH], in0=v3[:, :, 0:DH],
                        scalar1=maskc_sb[:, mc:mc + 1], scalar2=None,
                        op0=OP.mult,
                    )
                    nc.vector.tensor_scalar(
                        out=v3[:, :, DH:EH],
                        in0=ones8_sb[:].rearrange("p (h u) -> p h u", u=1),
                        scalar1=maskc_sb[:, mc:mc + 1], scalar2=None,
                        op0=OP.mult,
                    )


            # ---- phase 3: main n-chunk loop ----
            xt_r = xt_e.ap().rearrange("(cc p) n -> p cc n", p=128)  # [128,4,N]
            for ni in range(n_nc):
                n0 = ni * NCHUNK
                if ni == 0:
                    xt_sb = xt_pre
                else:
                    xt_sb = xtp.tile([128, 4, NCHUNK], bf16, tag="xt")
                    nc.sync.dma_start(
                        out=xt_sb[:], in_=xt_r[:, :, n0:n0 + NCHUNK])
                q_sb = qp.tile([128, 4, NCHUNK], bf16, tag="q")
                for ic in range(4):
                    ps = qpp.tile([128, NCHUNK], f32, tag="q")
                    for cc in range(4):
                        nc.tensor.matmul(
                            ps[:],
                            lhsT=r(wq_sb[:, cc, ic * 128:(ic + 1) * 128]),
                            rhs=r(xt_sb[:, cc, :]),
                            start=(cc == 0), stop=(cc == 3),
                        )
                    nc.vector.tensor_copy(out=q_sb[:, ic, :], in_=ps[:])
                q_sw = qp.tile([128, 4, NCHUNK], bf16, tag="qsw")
                nc.vector.tensor_copy(out=q_sw[0:64, :, :], in_=q_sb[64:128, :, :])
                nc.vector.tensor_copy(out=q_sw[64:128, :, :], in_=q_sb[0:64, :, :])
                o_sb = opool.tile([128, 4, NCHUNK], bf16, tag="o")
                o_st = opool.tile([EH, 8, NCHUNK], bf16, tag="ost")
                nA = min(3, nmc)
                nB = nmc - nA

                def scores_exp(h):
                    hc = h // 2
                    sA = spool.tile([128, 3, NCHUNK], f32, tag="sA")
                    sB = spool.tile([128, 2, NCHUNK], f32, tag="sB")
                    # mc even -> array rows 0-63, mc odd -> rows 64-127, so
                    # consecutive m-chunks run concurrently in the PE array.
                    for mc in range(nmc):
                        half = mc % 2
                        if (h % 2) == half:
                            ksrc, qsrc = k_sb, q_sb
                        else:
                            ksrc, qsrc = k_sw, q_sw
                        hp = half * 64
                        dst = sA[:, mc, :] if mc < nA else sB[:, mc - nA, :]
                        nc.tensor.matmul(
                            dst,
                            lhsT=r(ksrc[hp:hp + 64, hc, mc * 128:(mc + 1) * 128]),
                            rhs=r(qsrc[hp:hp + 64, hc, :]),
                            start=True, stop=True,
                        )
                    eA = ep.tile([128, 3, NCHUNK], bf16, tag="eA")
                    eB = ep.tile([128, 2, NCHUNK], bf16, tag="eB")
                    nc.scalar.activation(
                        out=eA[:, :nA, :], in_=sA[:, :nA, :], func=FT.Exp)
                    if nB:
                        nc.scalar.activation(
                            out=eB[:, :nB, :], in_=sB[:, :nB, :], func=FT.Exp)
                    return eA, eB

                def pv_drain(h, eA, eB):
                    pv = pvp.tile([EH, NCHUNK], f32, tag="pv")
                    for mc in range(nmc):
                        src = eA[:, mc, :] if mc < nA else eB[:, mc - nA, :]
                        nc.tensor.matmul(
                            pv[:],
                            lhsT=r(v2_sb[:, mc, h * EH:(h + 1) * EH]),
                            rhs=r(src),
                            start=(mc == 0), stop=(mc == nmc - 1),
                        )
                    nc.vector.tensor_copy(out=o_st[:, h, :], in_=pv[:])

                def half_divide(h):
                    if True:
                        g = h // 4
                        dT = r1p.tile([128, 4 * NCHUNK // 128], bf16,
                                      tag=f"dT{g}")
                        nc.gpsimd.dma_start(
                            out=dT[:],
                            in_=o_st[DH:EH, 4 * g:4 * g + 4, :].rearrange(
                                "p a b -> p (a b)"))
                        rT = r1p.tile([128, 4 * NCHUNK // 128], bf16,
                                      tag=f"rT{g}")
                        with nc.allow_low_precision("bf16 softmax denoms"):
                            nc.vector.reciprocal(out=rT[:], in_=dT[:])
                        rfl = r1p.tile([1, 4, NCHUNK], bf16, tag=f"rf{g}")
                        nc.gpsimd.dma_start(
                            out=rfl[:].rearrange("p a b -> p (a b)"),
                            in_=rT[:])
                        for hh in range(4 * g, 4 * g + 4):
                            rb = rbp.tile([64, NCHUNK], bf16, tag="rb")
                            nc.gpsimd.partition_broadcast(
                                out_ap=rb[:], in_ap=rfl[0:1, hh - 4 * g, :])
                            nc.vector.tensor_tensor(
                                out=o_sb[(hh % 2) * 64:(hh % 2) * 64 + 64,
                                         hh // 2, :],
                                in0=o_st[0:DH, hh, :], in1=rb[:],
                                op=OP.mult,
                            )

                prev = None
                for h in range(HEADS):
                    e_pair = scores_exp(h)
                    if prev is not None:
                        pv_drain(prev, *prev_e)
                        if prev == 3:
                            half_divide(3)
                    prev, prev_e = h, e_pair
                pv_drain(7, *prev_e)
                half_divide(7)
                for cc in range(4):
                    ps = fpp.tile([128, NCHUNK], f32, tag="fin")
                    for ic in range(4):
                        nc.tensor.matmul(
                            ps[:],
                            lhsT=r(wp_sb[:, ic, cc * 128:(cc + 1) * 128]),
                            rhs=r(o_sb[:, ic, :]),
                            start=(ic == 0), stop=(ic == 3),
                        )
                    y_sb = yp.tile([128, NCHUNK], f32, tag="y")
                    nc.vector.tensor_scalar(
                        out=y_sb[:], in0=ps[:], scalar1=bp_sb[:, cc:cc + 1],
                        scalar2=None, op0=OP.add,
                    )
                    nc.sync.dma_start(
                        out=out_e.ap()[cc * 128:(cc + 1) * 128, n0:n0 + NCHUNK],
                        in_=y_sb[:],
                    )

    nc.finalize()
    return nc


def _prep_inputs(x, mask, Wq, Wkv, conv_w, conv_b, ln_w, ln_b, Wp, bp, W):
    """Host-side sharding + layout prep. Returns (in_maps, m_pad)."""
    import ml_dtypes
    bf16 = ml_dtypes.bfloat16
    x = np.ascontiguousarray(np.asarray(x, dtype=np.float32))
    mask = np.asarray(mask, dtype=np.float32)
    Wq = np.asarray(Wq, dtype=np.float32)
    Wkv = np.asarray(Wkv, dtype=np.float32)
    conv_w = np.asarray(conv_w, dtype=np.float32)
    conv_b = np.asarray(conv_b, dtype=np.float32)
    ln_w = np.asarray(ln_w, dtype=np.float32)
    ln_b = np.asarray(ln_b, dtype=np.float32)
    Wp = np.asarray(Wp, dtype=np.float32)
    bp = np.asarray(bp, dtype=np.float32)

    Wm = W // SR
    kb = [int((mask[b] != 0).sum()) for b in range(B)]
    m_pad = max(256, ((max(kb) + 127) // 128) * 128)

    def rearr_w(w):  # [512, 512] -> [128, 4, 512] with [p, cc, :] = w[cc*128+p]
        return np.ascontiguousarray(w.reshape(4, 128, -1).transpose(1, 0, 2))

    def rearr_b(v):  # [512] -> [128, 4]
        return np.ascontiguousarray(v.reshape(4, 128).T)

    w2 = np.ascontiguousarray(
        conv_w.transpose(2, 3, 1, 0).reshape(4 * DIM, DIM)).astype(bf16)
    wq_in = rearr_w(Wq.T * np.float32(SCALE)).astype(bf16)
    wk_in = rearr_w((Wkv[:INNER] * ln_w).T).astype(bf16)
    wv_in = rearr_w((Wkv[INNER:] * ln_w).T).astype(bf16)
    wp_in = rearr_w(Wp.T).astype(bf16)
    bk_in = rearr_b(Wkv[:INNER] @ ln_b)
    bv_in = np.ascontiguousarray(Wkv[INNER:] @ ln_b)
    convb_in = rearr_b(conv_b)
    bp_in = rearr_b(bp)

    in_maps = []
    for b in range(B):
        xb = x[b]
        sel = np.nonzero(mask[b] != 0)[0]
        sel_pad = np.zeros(m_pad, dtype=np.int64)
        sel_pad[: len(sel)] = sel
        i = sel_pad // Wm
        j = sel_pad % Wm
        n_idx = np.stack(
            [(2 * i + di) * W + (2 * j + dj) for di in (0, 1) for dj in (0, 1)]
        )  # [4, m_pad], p = di*2+dj
        xp = xb[n_idx]  # [4, m_pad, 512]
        xp = np.ascontiguousarray(
            xp.transpose(0, 2, 1).reshape(4 * DIM, m_pad))
        maskc = (np.arange(m_pad) < len(sel)).astype(np.float32)
        maskc_in = np.ascontiguousarray(maskc.reshape(-1, 128).T)
        in_maps.append({
            "xt": np.ascontiguousarray(xb.T).astype(bf16),
            "xp": xp.astype(bf16),
            "w2": w2,
            "wq": wq_in,
            "wk": wk_in,
            "wv": wv_in,
            "wp": wp_in,
            "convb": convb_in,
            "bk": bk_in,
            "bv": bv_in,
            "bp": bp_in,
            "maskc": maskc_in,
        })
    return in_maps, m_pad


_BUILD_CACHE = {}


def kernel(x, H, W, mask, Wq, Wkv, conv_w, conv_b, ln_w, ln_b, Wp, bp,
           _results_hook=None):
    H = int(H)
    W = int(W)
    assert (H, W) == (64, 64) and x.shape == (B, N_SEQ, DIM), (H, W, x.shape)

    in_maps, m_pad = _prep_inputs(
        x, mask, Wq, Wkv, conv_w, conv_b, ln_w, ln_b, Wp, bp, W)

    if m_pad not in _BUILD_CACHE:
        _BUILD_CACHE[m_pad] = _build(m_pad)
    nc = _BUILD_CACHE[m_pad]

    _ensure_path()
    from concourse.bass_utils import run_bass_kernel_spmd

    res = run_bass_kernel_spmd(nc, in_maps, core_ids=list(range(B)))
    if _results_hook is not None:
        _results_hook(res)

    out = np.empty((B, N_SEQ, DIM), dtype=np.float32)
    for b in range(B):
        out[b] = res.results[b]["out"].T
    return out



# revision 13
# speedup vs baseline: 1.2282x; 1.2282x over previous
"""Spatial-reduction attention (PVT-style) on 8 TRN2 NeuronCores.

Strategy: pure data-parallel over batch (B=8 -> 1 batch element per core,
zero collectives). Per core, everything is computed in "feature-major"
(transposed) layout so that the attention-weight matrix E^T = exp(S^T)
lands with the context dim m on partitions -- exactly what the PV matmul
needs as its moving operand, so the big attention tensor is never
transposed on chip.

Key tricks (v3):
  - conv(stride 2, 2x2) == patch-merge matmul; patches are gathered
    host-side, only for the m positions with mask!=0 (mask compression,
    1024 -> m_pad 640), since masked context positions contribute
    nothing to the attention output.
  - mask + softmax denominator are folded into the PV matmul: the
    stationary operand V'' has 65 columns per head (64 = mask*V, 1 = mask),
    so row 64 of the PV output is the softmax denominator.
  - layernorm's ln_w/ln_b are folded into Wkv host-side; on-chip LN is a
    pure standardize using ones-matmul column stats + partition broadcast.
  - all matmuls run in bf16 (fp8 measured 8e-2 rel err -- the output has
    ~1/sqrt(Neff) signal shrinkage, so per-element noise is amplified
    ~12x; bf16 is the cheapest legal dtype).
  - the head loop is software-pipelined ACROSS n-chunks: one rolled
    64-iteration (chunk, head) loop; Q(c+1) is emitted at (c, h==2) and
    proj(c-1) at (c, h==4), so neither the PE nor the ACT exp stream
    stalls at chunk boundaries.
  - m pieces are (512, 128): the first 4 m-chunks (sA/eA, 4 PSUM banks)
    and the 5th (sB/eB, 1 bank) pipeline independently, and the piece-0
    prologue covers 4/5 of the context so attention can start early.
  - output is produced transposed ([512, 4096] per core) and untransposed
    on the host.
"""

import math
import numpy as np

N_SEQ = 4096
DIM = 512
HEADS = 8
DH = 64
INNER = HEADS * DH
SR = 2
SCALE = DH ** -0.5
LN_EPS = 1e-5
B = 8
NCHUNK = 512          # n-tile size of the main loop
EH = DH + 1           # 65: V'' columns per head (64 V + 1 mask/denominator)


def _ensure_path():
    try:
        import concourse.bass  # noqa: F401
    except ImportError:
        import sys
        for p in ("/opt/trn_rl_repo", "/root/.axon_site/_ro/trn_rl_repo"):
            if p not in sys.path:
                sys.path.append(p)


def _build(m_pad):
    _ensure_path()
    import concourse.bass as bass  # noqa: F401
    import concourse.mybir as mybir
    import concourse.tile as tile
    from concourse import bacc

    f32 = mybir.dt.float32
    bf16 = mybir.dt.bfloat16
    FT = mybir.ActivationFunctionType
    OP = mybir.AluOpType

    nmc = m_pad // 128
    assert nmc in (4, 5), nmc
    nA = min(4, nmc)
    nB = nmc - nA
    pieces = [(0, min(512, m_pad))]
    if m_pad > 512:
        pieces.append((512, m_pad))
    n_nc = N_SEQ // NCHUNK

    nc = bacc.Bacc()

    xt_e = nc.declare_dram_parameter("xt", [DIM, N_SEQ], bf16, isOutput=False)
    xp_e = nc.declare_dram_parameter("xp", [4 * DIM, m_pad], bf16, isOutput=False)
    w2_e = nc.declare_dram_parameter("w2", [4 * DIM, DIM], bf16, isOutput=False)
    wq_e = nc.declare_dram_parameter("wq", [128, 4, DIM], bf16, isOutput=False)
    wk_e = nc.declare_dram_parameter("wk", [128, 4, DIM], bf16, isOutput=False)
    wv_e = nc.declare_dram_parameter("wv", [128, 4, DIM], bf16, isOutput=False)
    wp_e = nc.declare_dram_parameter("wp", [128, 4, DIM], bf16, isOutput=False)
    convb_e = nc.declare_dram_parameter("convb", [128, 4], f32, isOutput=False)
    bk_e = nc.declare_dram_parameter("bk", [128, 4], f32, isOutput=False)
    bv_e = nc.declare_dram_parameter("bv", [DIM], f32, isOutput=False)
    bp_e = nc.declare_dram_parameter("bp", [128, 4], f32, isOutput=False)
    maskc_e = nc.declare_dram_parameter("maskc", [128, nmc], f32, isOutput=False)
    out_e = nc.declare_dram_parameter("out", [DIM, N_SEQ], f32, isOutput=True)

    def r(ap):
        return ap

    from contextlib import ExitStack

    with tile.TileContext(nc) as tc:
        with ExitStack() as stk:
            def pool(name, bufs, space="SBUF"):
                return stk.enter_context(
                    tc.tile_pool(name=name, bufs=bufs, space=space))

            wpool = pool("wts", 1)
            cpool = pool("consts", 1)
            xpcp = pool("stream", 1)
            w2p = pool("w2s", 1)
            ctxp = pool("ctx", 1)
            sqp = pool("sqs", 2)
            kvp = pool("kv", 1)
            xtp = pool("xtq", 2)
            qp = pool("qq", 2)
            ep = pool("ee", 2)
            opool = pool("oo", 2)
            yp = pool("yy", 3)
            smp = pool("small", 1)
            r1p = pool("r1p", 2)
            bcp = pool("bc", 1)
            rbp = pool("rbp", 2)
            spool = pool("ps_s", 1, space="PSUM")
            pvp = pool("ps_pv", 1, space="PSUM")
            qpp = pool("ps_q", 1, space="PSUM")
            fpp = pool("ps_f", 1, space="PSUM")

            # ---- constants / weights (resident) ----
            wq_sb = wpool.tile([128, 4, DIM], bf16, tag="wq")
            nc.gpsimd.dma_start(out=wq_sb[:], in_=wq_e.ap())
            wk_sb = wpool.tile([128, 4, DIM], bf16, tag="wk")
            nc.gpsimd.dma_start(out=wk_sb[:], in_=wk_e.ap())
            wv_sb = wpool.tile([128, 4, DIM], bf16, tag="wv")
            nc.gpsimd.dma_start(out=wv_sb[:], in_=wv_e.ap())
            wp_sb = wpool.tile([128, 4, DIM], bf16, tag="wp")
            nc.gpsimd.dma_start(out=wp_sb[:], in_=wp_e.ap())
            convb_sb = cpool.tile([128, 4], f32, tag="convb")
            nc.gpsimd.dma_start(out=convb_sb[:], in_=convb_e.ap())
            bk_sb = cpool.tile([128, 4], f32, tag="bk")
            nc.gpsimd.dma_start(out=bk_sb[:], in_=bk_e.ap())
            bp_sb = cpool.tile([128, 4], f32, tag="bp")
            nc.gpsimd.dma_start(out=bp_sb[:], in_=bp_e.ap())
            maskc_sb = cpool.tile([128, nmc], f32, tag="maskc")
            nc.gpsimd.dma_start(out=maskc_sb[:], in_=maskc_e.ap())
            bv_bc = cpool.tile([128, DIM], f32, tag="bvbc")
            nc.gpsimd.dma_start(out=bv_bc[:], in_=bv_e.ap().partition_broadcast(128))
            ones_sb = cpool.tile([128, 1], bf16, tag="ones")
            nc.vector.memset(ones_sb[:], 1.0)
            ones8_sb = cpool.tile([128, 8], f32, tag="ones8")
            nc.vector.memset(ones8_sb[:], 1.0)
            eps_sb = cpool.tile([1, 1], f32, tag="eps")
            nc.vector.memset(eps_sb[:], LN_EPS)

            # xt chunk-0 prefetch, issued before the bulk xp/w2 stream
            xt_r = xt_e.ap().rearrange("(cc p) n -> p cc n", p=128)
            xt_pre = xtp.tile([128, 4, NCHUNK], bf16, tag="xt")
            nc.sync.dma_start(out=xt_pre[:], in_=xt_r[:, :, 0:NCHUNK])

            xp_r = xp_e.ap().rearrange("(kc p) m -> kc p m", p=128)
            w2_r = w2_e.ap().rearrange("(kc p) co -> kc p co", p=128)
            xp_t = []
            w2_t = []
            for kc in range(16):
                xpk = xpcp.tile([128, m_pad], bf16, tag=f"xp{kc}")
                nc.sync.dma_start(out=xpk[:], in_=xp_r[kc, :, :])
                xp_t.append(xpk)
                w2k = w2p.tile([128, DIM], bf16, tag=f"w2{kc}")
                nc.sync.dma_start(out=w2k[:], in_=w2_r[kc, :, :])
                w2_t.append(w2k)

            # ---- Q projection (chunk c) ----
            def emit_q(c):
                n0 = c * NCHUNK
                if c == 0:
                    xt_sb = xt_pre
                else:
                    xt_sb = xtp.tile([128, 4, NCHUNK], bf16, tag="xt")
                    nc.sync.dma_start(
                        out=xt_sb[:], in_=xt_r[:, :, n0:n0 + NCHUNK])
                q_sb = qp.tile([128, 4, NCHUNK], bf16, tag="q")
                for ic in range(4):
                    ps = qpp.tile([128, NCHUNK], f32, tag="q")
                    for cc in range(4):
                        nc.tensor.matmul(
                            ps[:],
                            lhsT=r(wq_sb[:, cc, ic * 128:(ic + 1) * 128]),
                            rhs=r(xt_sb[:, cc, :]),
                            start=(cc == 0), stop=(cc == 3),
                        )
                    nc.vector.tensor_copy(out=q_sb[:, ic, :], in_=ps[:])
                q_sw = qp.tile([128, 4, NCHUNK], bf16, tag="qsw")
                nc.vector.tensor_copy(out=q_sw[0:64, :, :], in_=q_sb[64:128, :, :])
                nc.vector.tensor_copy(out=q_sw[64:128, :, :], in_=q_sb[0:64, :, :])
                return q_sb, q_sw

            # Q(0) early: only needs xt chunk 0, warms the PE while the
            # patch stream is still loading.
            q_cur = emit_q(0)

            # ---- phase 1: patch-merge ctx^T + layernorm + K/V, per piece ----
            ctx_raw = ctxp.tile([128, 4, m_pad], bf16, tag="craw")
            ctxn = ctxp.tile([128, 4, m_pad], bf16, tag="cn")
            k_sb = kvp.tile([128, 4, m_pad], bf16, tag="k")
            k_sw = kvp.tile([128, 4, m_pad], bf16, tag="ksw")
            v2_sb = kvp.tile([128, nmc, HEADS * EH], bf16, tag="v2")
            bv3 = bv_bc[:].rearrange("p (h d) -> p h d", d=DH)

            for (p0, p1) in pieces:
                pw = p1 - p0
                psA = spool.tile([128, 4, NCHUNK], f32, tag="sA")
                for kc in range(16):
                    for cco in range(4):
                        nc.tensor.matmul(
                            psA[:, cco, :pw],
                            lhsT=r(w2_t[kc][:, cco * 128:(cco + 1) * 128]),
                            rhs=r(xp_t[kc][:, p0:p1]),
                            start=(kc == 0), stop=(kc == 15),
                        )
                for cco in range(4):
                    nc.scalar.activation(
                        out=ctx_raw[:, cco, p0:p1], in_=psA[:, cco, :pw],
                        func=FT.Identity, bias=convb_sb[:, cco:cco + 1],
                    )
                # column stats via ones-matmul (sum over the c partition dim)
                mu_ps = pvp.tile([EH, NCHUNK], f32, tag="pv")
                ss_ps = fpp.tile([128, NCHUNK], f32, tag="fin")
                for cc in range(4):
                    sq_s = sqp.tile([128, NCHUNK], bf16, tag="sqs")
                    nc.scalar.activation(
                        out=sq_s[:, :pw], in_=ctx_raw[:, cc, p0:p1],
                        func=FT.Square,
                    )
                    nc.tensor.matmul(
                        mu_ps[0:1, :pw], lhsT=r(ones_sb[:]),
                        rhs=r(ctx_raw[:, cc, p0:p1]),
                        start=(cc == 0), stop=(cc == 3),
                    )
                    nc.tensor.matmul(
                        ss_ps[0:1, :pw], lhsT=r(ones_sb[:]),
                        rhs=r(sq_s[:, :pw]),
                        start=(cc == 0), stop=(cc == 3),
                    )
                m1n = smp.tile([1, NCHUNK], f32, tag="m1n")
                nc.vector.tensor_scalar(
                    out=m1n[:, :pw], in0=mu_ps[0:1, :pw],
                    scalar1=-1.0 / DIM, scalar2=None, op0=OP.mult,
                )
                v1 = smp.tile([1, NCHUNK], f32, tag="v1")
                nc.vector.tensor_scalar(
                    out=v1[:, :pw], in0=ss_ps[0:1, :pw],
                    scalar1=1.0 / DIM, scalar2=None, op0=OP.mult,
                )
                m2 = smp.tile([1, NCHUNK], f32, tag="m2")
                nc.vector.tensor_tensor(
                    out=m2[:, :pw], in0=m1n[:, :pw], in1=m1n[:, :pw], op=OP.mult
                )
                var = smp.tile([1, NCHUNK], f32, tag="var")
                nc.vector.tensor_tensor(
                    out=var[:, :pw], in0=v1[:, :pw], in1=m2[:, :pw], op=OP.subtract
                )
                std = smp.tile([1, NCHUNK], f32, tag="std")
                nc.scalar.activation(
                    out=std[:, :pw], in_=var[:, :pw], func=FT.Sqrt,
                    bias=eps_sb[:],
                )
                rstd = smp.tile([1, NCHUNK], f32, tag="rstd")
                nc.vector.reciprocal(out=rstd[:, :pw], in_=std[:, :pw])
                tsh = smp.tile([1, NCHUNK], f32, tag="tsh")
                nc.vector.tensor_tensor(
                    out=tsh[:, :pw], in0=m1n[:, :pw], in1=rstd[:, :pw], op=OP.mult
                )
                r_bc = bcp.tile([128, NCHUNK], f32, tag="rbc")
                nc.gpsimd.partition_broadcast(out_ap=r_bc[:, :pw], in_ap=rstd[:, :pw])
                t_bc = bcp.tile([128, NCHUNK], f32, tag="tbc")
                nc.gpsimd.partition_broadcast(out_ap=t_bc[:, :pw], in_ap=tsh[:, :pw])
                for cc in range(4):
                    nc.vector.tensor_tensor(
                        out=ctxn[:, cc, p0:p1], in0=ctx_raw[:, cc, p0:p1],
                        in1=r_bc[:, :pw], op=OP.mult,
                    )
                    nc.vector.tensor_tensor(
                        out=ctxn[:, cc, p0:p1], in0=ctxn[:, cc, p0:p1],
                        in1=t_bc[:, :pw], op=OP.add,
                    )
                # K^T (feature-major) for this piece
                for kc in range(4):
                    ps = qpp.tile([128, NCHUNK], f32, tag="q")
                    for cc in range(4):
                        nc.tensor.matmul(
                            ps[:, :pw],
                            lhsT=r(wk_sb[:, cc, kc * 128:(kc + 1) * 128]),
                            rhs=r(ctxn[:, cc, p0:p1]),
                            start=(cc == 0), stop=(cc == 3),
                        )
                    nc.scalar.activation(
                        out=k_sb[:, kc, p0:p1], in_=ps[:, :pw],
                        func=FT.Identity, bias=bk_sb[:, kc:kc + 1],
                    )
                nc.vector.tensor_copy(
                    out=k_sw[0:64, :, p0:p1], in_=k_sb[64:128, :, p0:p1])
                nc.vector.tensor_copy(
                    out=k_sw[64:128, :, p0:p1], in_=k_sb[0:64, :, p0:p1])
                # V'' (token-major) for this piece's m-chunks
                for mc in range(p0 // 128, p1 // 128):
                    ps = fpp.tile([128, NCHUNK], f32, tag="fin")
                    for cc in range(4):
                        nc.tensor.matmul(
                            ps[:],
                            lhsT=r(ctxn[:, cc, mc * 128:(mc + 1) * 128]),
                            rhs=r(wv_sb[:, cc, :]),
                            start=(cc == 0), stop=(cc == 3),
                        )
                    v3 = v2_sb[:, mc, :].rearrange("p (h e) -> p h e", e=EH)
                    nc.vector.tensor_tensor(
                        out=v3[:, :, 0:DH],
                        in0=ps[:].rearrange("p (h d) -> p h d", d=DH),
                        in1=bv3, op=OP.add,
                    )
                    nc.vector.tensor_scalar(
                        out=v3[:, :, 0:DH], in0=v3[:, :, 0:DH],
                        scalar1=maskc_sb[:, mc:mc + 1], scalar2=None,
                        op0=OP.mult,
                    )
                    nc.vector.tensor_scalar(
                        out=v3[:, :, DH:EH],
                        in0=ones8_sb[:].rearrange("p (h u) -> p h u", u=1),
                        scalar1=maskc_sb[:, mc:mc + 1], scalar2=None,
                        op0=OP.mult,
                    )

            # ---- phase 2: rolled main loop over (chunk, head) ----
            def scores_exp(h, q_sb, q_sw):
                sA = spool.tile([128, 4, NCHUNK], f32, tag="sA", name="sA")
                sB = None
                if nB:
                    sB = spool.tile([128, 1, NCHUNK], f32, tag="sB", name="sB")
                # mc even -> array rows 0-63, mc odd -> rows 64-127, so
                # consecutive m-chunks run concurrently in the PE array.
                for mc in range(nmc):
                    half = mc % 2
                    if (h % 2) == half:
                        ksrc, qsrc = k_sb, q_sb
                    else:
                        ksrc, qsrc = k_sw, q_sw
                    hp = half * 64
                    hc = h // 2
                    dst = sA[:, mc, :] if mc < nA else sB[:, mc - nA, :]
                    nc.tensor.matmul(
                        dst,
                        lhsT=r(ksrc[hp:hp + 64, hc, mc * 128:(mc + 1) * 128]),
                        rhs=r(qsrc[hp:hp + 64, hc, :]),
                        start=True, stop=True,
                    )
                eA = ep.tile([128, nA, NCHUNK], bf16, tag="eA")
                nc.scalar.activation(
                    out=eA[:], in_=sA[:, :nA, :], func=FT.Exp)
                eB = None
                if nB:
                    eB = ep.tile([128, nB, NCHUNK], bf16, tag="eB")
                    nc.scalar.activation(
                        out=eB[:], in_=sB[:, :nB, :], func=FT.Exp)
                return eA, eB

            def pv_drain(h, eA, eB, o_st):
                pv = pvp.tile([EH, NCHUNK], f32, tag="pv")
                for mc in range(nmc):
                    src = eA[:, mc, :] if mc < nA else eB[:, mc - nA, :]
                    nc.tensor.matmul(
                        pv[:],
                        lhsT=r(v2_sb[:, mc, h * EH:(h + 1) * EH]),
                        rhs=r(src),
                        start=(mc == 0), stop=(mc == nmc - 1),
                    )
                nc.vector.tensor_copy(out=o_st[:, h, :], in_=pv[:])

            def half_divide(g4, o_st, o_sb):
                dT = r1p.tile([128, 4 * NCHUNK // 128], bf16, tag=f"dT{g4}")
                nc.gpsimd.dma_start(
                    out=dT[:],
                    in_=o_st[DH:EH, 4 * g4:4 * g4 + 4, :].rearrange(
                        "p a b -> p (a b)"))
                rT = r1p.tile([128, 4 * NCHUNK // 128], bf16, tag=f"rT{g4}")
                with nc.allow_low_precision("bf16 softmax denoms"):
                    nc.vector.reciprocal(out=rT[:], in_=dT[:])
                rfl = r1p.tile([1, 4, NCHUNK], bf16, tag=f"rf{g4}")
                nc.gpsimd.dma_start(
                    out=rfl[:].rearrange("p a b -> p (a b)"),
                    in_=rT[:])
                for hh in range(4 * g4, 4 * g4 + 4):
                    rb = rbp.tile([64, NCHUNK], bf16, tag="rb")
                    nc.gpsimd.partition_broadcast(
                        out_ap=rb[:], in_ap=rfl[0:1, hh - 4 * g4, :])
                    nc.vector.tensor_tensor(
                        out=o_sb[(hh % 2) * 64:(hh % 2) * 64 + 64,
                                 hh // 2, :],
                        in0=o_st[0:DH, hh, :], in1=rb[:],
                        op=OP.mult,
                    )

            def emit_proj(c, o_sb):
                n0 = c * NCHUNK
                for cc in range(4):
                    ps = fpp.tile([128, NCHUNK], f32, tag="fin")
                    for ic in range(4):
                        nc.tensor.matmul(
                            ps[:],
                            lhsT=r(wp_sb[:, ic, cc * 128:(cc + 1) * 128]),
                            rhs=r(o_sb[:, ic, :]),
                            start=(ic == 0), stop=(ic == 3),
                        )
                    y_sb = yp.tile([128, NCHUNK], f32, tag="y")
                    nc.vector.tensor_scalar(
                        out=y_sb[:], in0=ps[:], scalar1=bp_sb[:, cc:cc + 1],
                        scalar2=None, op0=OP.add,
                    )
                    nc.sync.dma_start(
                        out=out_e.ap()[cc * 128:(cc + 1) * 128, n0:n0 + NCHUNK],
                        in_=y_sb[:],
                    )

            q_next = None
            o_sb_by_c = {}
            o_st_by_c = {}
            prev = None  # (c, h, eA, eB)
            for g in range(n_nc * HEADS):
                c, h = divmod(g, HEADS)
                if h == 0:
                    o_sb_by_c[c] = opool.tile(
                        [128, 4, NCHUNK], bf16, tag="o", name="o_sb")
                    o_st_by_c[c] = opool.tile(
                        [EH, 8, NCHUNK], bf16, tag="ost", name="o_st")
                e_pair = scores_exp(h, *q_cur)
                if prev is not None:
                    pc, ph, peA, peB = prev
                    pv_drain(ph, peA, peB, o_st_by_c[pc])
                    if ph == 3:
                        half_divide(0, o_st_by_c[pc], o_sb_by_c[pc])
                    elif ph == 7:
                        half_divide(1, o_st_by_c[pc], o_sb_by_c[pc])
                prev = (c, h, *e_pair)
                if h == 2 and c + 1 < n_nc:
                    q_next = emit_q(c + 1)
                elif h == 4 and c > 0:
                    emit_proj(c - 1, o_sb_by_c.pop(c - 1))
                    o_st_by_c.pop(c - 1)
                elif h == 7:
                    q_cur = q_next
            pc, ph, peA, peB = prev
            pv_drain(ph, peA, peB, o_st_by_c[pc])
            half_divide(1, o_st_by_c[pc], o_sb_by_c[pc])
            emit_proj(n_nc - 1, o_sb_by_c.pop(n_nc - 1))

    nc.finalize()
    return nc


def _prep_inputs(x, mask, Wq, Wkv, conv_w, conv_b, ln_w, ln_b, Wp, bp, W):
    """Host-side sharding + layout prep. Returns (in_maps, m_pad)."""
    import ml_dtypes
    bf16 = ml_dtypes.bfloat16
    x = np.ascontiguousarray(np.asarray(x, dtype=np.float32))
    mask = np.asarray(mask, dtype=np.float32)
    Wq = np.asarray(Wq, dtype=np.float32)
    Wkv = np.asarray(Wkv, dtype=np.float32)
    conv_w = np.asarray(conv_w, dtype=np.float32)
    conv_b = np.asarray(conv_b, dtype=np.float32)
    ln_w = np.asarray(ln_w, dtype=np.float32)
    ln_b = np.asarray(ln_b, dtype=np.float32)
    Wp = np.asarray(Wp, dtype=np.float32)
    bp = np.asarray(bp, dtype=np.float32)

    Wm = W // SR
    kb = [int((mask[b] != 0).sum()) for b in range(B)]
    m_pad = max(512, ((max(kb) + 127) // 128) * 128)

    def rearr_w(w):  # [512, 512] -> [128, 4, 512] with [p, cc, :] = w[cc*128+p]
        return np.ascontiguousarray(w.reshape(4, 128, -1).transpose(1, 0, 2))

    def rearr_b(v):  # [512] -> [128, 4]
        return np.ascontiguousarray(v.reshape(4, 128).T)

    w2 = np.ascontiguousarray(
        conv_w.transpose(2, 3, 1, 0).reshape(4 * DIM, DIM)).astype(bf16)
    wq_in = rearr_w(Wq.T * np.float32(SCALE)).astype(bf16)
    wk_in = rearr_w((Wkv[:INNER] * ln_w).T).astype(bf16)
    wv_in = rearr_w((Wkv[INNER:] * ln_w).T).astype(bf16)
    wp_in = rearr_w(Wp.T).astype(bf16)
    bk_in = rearr_b(Wkv[:INNER] @ ln_b)
    bv_in = np.ascontiguousarray(Wkv[INNER:] @ ln_b)
    convb_in = rearr_b(conv_b)
    bp_in = rearr_b(bp)

    in_maps = []
    for b in range(B):
        xb = x[b]
        sel = np.nonzero(mask[b] != 0)[0]
        sel_pad = np.zeros(m_pad, dtype=np.int64)
        sel_pad[: len(sel)] = sel
        i = sel_pad // Wm
        j = sel_pad % Wm
        n_idx = np.stack(
            [(2 * i + di) * W + (2 * j + dj) for di in (0, 1) for dj in (0, 1)]
        )  # [4, m_pad], p = di*2+dj
        xp = xb[n_idx]  # [4, m_pad, 512]
        xp = np.ascontiguousarray(
            xp.transpose(0, 2, 1).reshape(4 * DIM, m_pad))
        maskc = (np.arange(m_pad) < len(sel)).astype(np.float32)
        maskc_in = np.ascontiguousarray(maskc.reshape(-1, 128).T)
        in_maps.append({
            "xt": np.ascontiguousarray(xb.T).astype(bf16),
            "xp": xp.astype(bf16),
            "w2": w2,
            "wq": wq_in,
            "wk": wk_in,
            "wv": wv_in,
            "wp": wp_in,
            "convb": convb_in,
            "bk": bk_in,
            "bv": bv_in,
            "bp": bp_in,
            "maskc": maskc_in,
        })
    return in_maps, m_pad


_BUILD_CACHE = {}


def kernel(x, H, W, mask, Wq, Wkv, conv_w, conv_b, ln_w, ln_b, Wp, bp,
           _results_hook=None):
    H = int(H)
    W = int(W)
    assert (H, W) == (64, 64) and x.shape == (B, N_SEQ, DIM), (H, W, x.shape)

    in_maps, m_pad = _prep_inputs(
        x, mask, Wq, Wkv, conv_w, conv_b, ln_w, ln_b, Wp, bp, W)

    if m_pad not in _BUILD_CACHE:
        _BUILD_CACHE[m_pad] = _build(m_pad)
    nc = _BUILD_CACHE[m_pad]

    _ensure_path()
    from concourse.bass_utils import run_bass_kernel_spmd

    res = run_bass_kernel_spmd(nc, in_maps, core_ids=list(range(B)))
    if _results_hook is not None:
        _results_hook(res)

    out = np.empty((B, N_SEQ, DIM), dtype=np.float32)
    for b in range(B):
        out[b] = res.results[b]["out"].T
    return out


# revision 18
# speedup vs baseline: 1.2491x; 1.0170x over previous
"""Spatial-reduction attention (PVT-style) on 8 TRN2 NeuronCores.

Strategy: pure data-parallel over batch (B=8 -> 1 batch element per core,
zero collectives). Per core, everything is computed in "feature-major"
(transposed) layout so that the attention-weight matrix E^T = exp(S^T)
lands with the context dim m on partitions -- exactly what the PV matmul
needs as its moving operand, so the big attention tensor is never
transposed on chip.

Key tricks (v3):
  - conv(stride 2, 2x2) == patch-merge matmul; patches are gathered
    host-side, only for the m positions with mask!=0 (mask compression,
    1024 -> m_pad 640), since masked context positions contribute
    nothing to the attention output.
  - mask + softmax denominator are folded into the PV matmul: the
    stationary operand V'' has 65 columns per head (64 = mask*V, 1 = mask),
    so row 64 of the PV output is the softmax denominator.
  - layernorm's ln_w/ln_b are folded into Wkv host-side; on-chip LN is a
    pure standardize using ones-matmul column stats + partition broadcast.
  - all matmuls run in bf16 (fp8 measured 8e-2 rel err -- the output has
    ~1/sqrt(Neff) signal shrinkage, so per-element noise is amplified
    ~12x; bf16 is the cheapest legal dtype).
  - the head loop is software-pipelined ACROSS n-chunks: one rolled
    64-iteration (chunk, head) loop; Q(c+1) is emitted at (c, h==2) and
    proj(c-1) at (c, h==4), so neither the PE nor the ACT exp stream
    stalls at chunk boundaries.
  - m pieces are (512, 128): the first 4 m-chunks (sA/eA, 4 PSUM banks)
    and the 5th (sB/eB, 1 bank) pipeline independently, and the piece-0
    prologue covers 4/5 of the context so attention can start early.
  - output is produced transposed ([512, 4096] per core) and untransposed
    on the host.
"""

import math
import numpy as np

N_SEQ = 4096
DIM = 512
HEADS = 8
DH = 64
INNER = HEADS * DH
SR = 2
SCALE = DH ** -0.5
LN_EPS = 1e-5
B = 8
NCHUNK = 512          # n-tile size of the main loop
EH = DH + 1           # 65: V'' columns per head (64 V + 1 mask/denominator)


def _ensure_path():
    try:
        import concourse.bass  # noqa: F401
    except ImportError:
        import sys
        for p in ("/opt/trn_rl_repo", "/root/.axon_site/_ro/trn_rl_repo"):
            if p not in sys.path:
                sys.path.append(p)


def _build(m_pad):
    _ensure_path()
    import concourse.bass as bass  # noqa: F401
    import concourse.mybir as mybir
    import concourse.tile as tile
    from concourse import bacc

    f32 = mybir.dt.float32
    bf16 = mybir.dt.bfloat16
    FT = mybir.ActivationFunctionType
    OP = mybir.AluOpType

    nmc = m_pad // 128
    assert nmc in (4, 5), nmc
    nA = min(4, nmc)
    nB = nmc - nA
    pieces = [(0, min(512, m_pad))]
    if m_pad > 512:
        pieces.append((512, m_pad))
    n_nc = N_SEQ // NCHUNK

    nc = bacc.Bacc()

    xt_e = nc.declare_dram_parameter("xt", [DIM, N_SEQ], bf16, isOutput=False)
    xp_e = nc.declare_dram_parameter("xp", [4 * DIM, m_pad], bf16, isOutput=False)
    w2_e = nc.declare_dram_parameter("w2", [4 * DIM, DIM], bf16, isOutput=False)
    wq_e = nc.declare_dram_parameter("wq", [128, 4, DIM], bf16, isOutput=False)
    wk_e = nc.declare_dram_parameter("wk", [128, 4, DIM], bf16, isOutput=False)
    wv_e = nc.declare_dram_parameter("wv", [128, 4, DIM], bf16, isOutput=False)
    wp_e = nc.declare_dram_parameter("wp", [128, 4, DIM], bf16, isOutput=False)
    convb_e = nc.declare_dram_parameter("convb", [128, 4], f32, isOutput=False)
    bk_e = nc.declare_dram_parameter("bk", [128, 4], f32, isOutput=False)
    bv_e = nc.declare_dram_parameter("bv", [DIM], f32, isOutput=False)
    bp_e = nc.declare_dram_parameter("bp", [128, 4], f32, isOutput=False)
    maskc_e = nc.declare_dram_parameter("maskc", [128, nmc], f32, isOutput=False)
    out_e = nc.declare_dram_parameter("out", [DIM, N_SEQ], f32, isOutput=True)

    def r(ap):
        return ap

    from contextlib import ExitStack

    with tile.TileContext(nc) as tc:
        with ExitStack() as stk:
            def pool(name, bufs, space="SBUF"):
                return stk.enter_context(
                    tc.tile_pool(name=name, bufs=bufs, space=space))

            wpool = pool("wts", 1)
            cpool = pool("consts", 1)
            xpcp = pool("stream", 1)
            w2p = pool("w2s", 1)
            ctxp = pool("ctx", 1)
            sqp = pool("sqs", 2)
            kvp = pool("kv", 1)
            xtp = pool("xtq", 2)
            qp = pool("qq", 2)
            ep = pool("ee", 2)
            opool = pool("oo", 2)
            yp = pool("yy", 3)
            smp = pool("small", 1)
            r1p = pool("r1p", 2)
            bcp = pool("bc", 1)
            rbp = pool("rbp", 2)
            spool = pool("ps_s", 1, space="PSUM")
            pvp = pool("ps_pv", 1, space="PSUM")
            qpp = pool("ps_q", 1, space="PSUM")
            fpp = pool("ps_f", 1, space="PSUM")

            # ---- constants / weights (resident) ----
            wq_sb = wpool.tile([128, 4, DIM], bf16, tag="wq")
            nc.gpsimd.dma_start(out=wq_sb[:], in_=wq_e.ap())
            wk_sb = wpool.tile([128, 4, DIM], bf16, tag="wk")
            nc.gpsimd.dma_start(out=wk_sb[:], in_=wk_e.ap())
            wv_sb = wpool.tile([128, 4, DIM], bf16, tag="wv")
            nc.gpsimd.dma_start(out=wv_sb[:], in_=wv_e.ap())
            wp_sb = wpool.tile([128, 4, DIM], bf16, tag="wp")
            nc.gpsimd.dma_start(out=wp_sb[:], in_=wp_e.ap())
            convb_sb = cpool.tile([128, 4], f32, tag="convb")
            nc.gpsimd.dma_start(out=convb_sb[:], in_=convb_e.ap())
            bk_sb = cpool.tile([128, 4], f32, tag="bk")
            nc.gpsimd.dma_start(out=bk_sb[:], in_=bk_e.ap())
            bp_sb = cpool.tile([128, 4], f32, tag="bp")
            nc.gpsimd.dma_start(out=bp_sb[:], in_=bp_e.ap())
            maskc_sb = cpool.tile([128, nmc], f32, tag="maskc")
            nc.gpsimd.dma_start(out=maskc_sb[:], in_=maskc_e.ap())
            bv_bc = cpool.tile([128, DIM], f32, tag="bvbc")
            nc.gpsimd.dma_start(out=bv_bc[:], in_=bv_e.ap().partition_broadcast(128))
            ones_sb = cpool.tile([128, 1], bf16, tag="ones")
            nc.vector.memset(ones_sb[:], 1.0)
            ones8_sb = cpool.tile([128, 8], f32, tag="ones8")
            nc.vector.memset(ones8_sb[:], 1.0)
            eps_sb = cpool.tile([1, 1], f32, tag="eps")
            nc.vector.memset(eps_sb[:], LN_EPS)

            # xt chunk-0 prefetch, issued before the bulk xp/w2 stream
            xt_r = xt_e.ap().rearrange("(cc p) n -> p cc n", p=128)
            xt_pre = xtp.tile([128, 4, NCHUNK], bf16, tag="xt")
            nc.sync.dma_start(out=xt_pre[:], in_=xt_r[:, :, 0:NCHUNK])

            xp_r = xp_e.ap().rearrange("(kc p) m -> kc p m", p=128)
            w2_r = w2_e.ap().rearrange("(kc p) co -> kc p co", p=128)
            xp_t = []
            w2_t = []
            for kc in range(16):
                xpk = xpcp.tile([128, m_pad], bf16, tag=f"xp{kc}")
                nc.sync.dma_start(out=xpk[:], in_=xp_r[kc, :, :])
                xp_t.append(xpk)
                w2k = w2p.tile([128, DIM], bf16, tag=f"w2{kc}")
                nc.sync.dma_start(out=w2k[:], in_=w2_r[kc, :, :])
                w2_t.append(w2k)

            # ---- Q projection (chunk c) ----
            def emit_q(c):
                n0 = c * NCHUNK
                if c == 0:
                    xt_sb = xt_pre
                else:
                    xt_sb = xtp.tile([128, 4, NCHUNK], bf16, tag="xt")
                    nc.sync.dma_start(
                        out=xt_sb[:], in_=xt_r[:, :, n0:n0 + NCHUNK])
                q_sb = qp.tile([128, 4, NCHUNK], bf16, tag="q")
                for ic in range(4):
                    ps = qpp.tile([128, NCHUNK], f32, tag="q")
                    for cc in range(4):
                        nc.tensor.matmul(
                            ps[:],
                            lhsT=r(wq_sb[:, cc, ic * 128:(ic + 1) * 128]),
                            rhs=r(xt_sb[:, cc, :]),
                            start=(cc == 0), stop=(cc == 3),
                        )
                    nc.vector.tensor_copy(out=q_sb[:, ic, :], in_=ps[:])
                q_sw = qp.tile([128, 4, NCHUNK], bf16, tag="qsw")
                nc.vector.tensor_copy(out=q_sw[0:64, :, :], in_=q_sb[64:128, :, :])
                nc.vector.tensor_copy(out=q_sw[64:128, :, :], in_=q_sb[0:64, :, :])
                return q_sb, q_sw

            # Q(0) early: only needs xt chunk 0, warms the PE while the
            # patch stream is still loading.
            q_cur = emit_q(0)

            # ---- phase 1: patch-merge ctx^T + layernorm + K/V, per piece ----
            ctx_raw = ctxp.tile([128, 4, m_pad], bf16, tag="craw")
            ctxn = ctxp.tile([128, 4, m_pad], bf16, tag="cn")
            k_sb = kvp.tile([128, 4, m_pad], bf16, tag="k")
            k_sw = kvp.tile([128, 4, m_pad], bf16, tag="ksw")
            v2_sb = kvp.tile([128, nmc, HEADS * EH], bf16, tag="v2")
            bv3 = bv_bc[:].rearrange("p (h d) -> p h d", d=DH)

            for (p0, p1) in pieces:
                pw = p1 - p0
                psA = spool.tile([128, 4, NCHUNK], f32, tag="sA")
                for kc in range(16):
                    for cco in range(4):
                        nc.tensor.matmul(
                            psA[:, cco, :pw],
                            lhsT=r(w2_t[kc][:, cco * 128:(cco + 1) * 128]),
                            rhs=r(xp_t[kc][:, p0:p1]),
                            start=(kc == 0), stop=(kc == 15),
                        )
                for cco in range(4):
                    nc.scalar.activation(
                        out=ctx_raw[:, cco, p0:p1], in_=psA[:, cco, :pw],
                        func=FT.Identity, bias=convb_sb[:, cco:cco + 1],
                    )
                # column stats via ones-matmul (sum over the c partition dim)
                mu_ps = pvp.tile([EH, NCHUNK], f32, tag="pv")
                ss_ps = fpp.tile([128, NCHUNK], f32, tag="fin")
                for cc in range(4):
                    sq_s = sqp.tile([128, NCHUNK], bf16, tag="sqs")
                    nc.vector.tensor_tensor(
                        out=sq_s[:, :pw], in0=ctx_raw[:, cc, p0:p1],
                        in1=ctx_raw[:, cc, p0:p1], op=OP.mult,
                    )
                    nc.tensor.matmul(
                        mu_ps[0:1, :pw], lhsT=r(ones_sb[:]),
                        rhs=r(ctx_raw[:, cc, p0:p1]),
                        start=(cc == 0), stop=(cc == 3),
                    )
                    nc.tensor.matmul(
                        ss_ps[0:1, :pw], lhsT=r(ones_sb[:]),
                        rhs=r(sq_s[:, :pw]),
                        start=(cc == 0), stop=(cc == 3),
                    )
                m1n = smp.tile([1, NCHUNK], f32, tag="m1n")
                nc.vector.tensor_scalar(
                    out=m1n[:, :pw], in0=mu_ps[0:1, :pw],
                    scalar1=-1.0 / DIM, scalar2=None, op0=OP.mult,
                )
                v1 = smp.tile([1, NCHUNK], f32, tag="v1")
                nc.vector.tensor_scalar(
                    out=v1[:, :pw], in0=ss_ps[0:1, :pw],
                    scalar1=1.0 / DIM, scalar2=None, op0=OP.mult,
                )
                m2 = smp.tile([1, NCHUNK], f32, tag="m2")
                nc.vector.tensor_tensor(
                    out=m2[:, :pw], in0=m1n[:, :pw], in1=m1n[:, :pw], op=OP.mult
                )
                var = smp.tile([1, NCHUNK], f32, tag="var")
                nc.vector.tensor_tensor(
                    out=var[:, :pw], in0=v1[:, :pw], in1=m2[:, :pw], op=OP.subtract
                )
                # rstd = exp(-0.5*ln(var+eps)): Ln/Exp share one ACT table
                # set (no sqrt-set load) and beat DVE reciprocal's ~6.5
                # cycles/elem on a 1-partition operand.
                lv = smp.tile([1, NCHUNK], f32, tag="lv")
                nc.scalar.activation(
                    out=lv[:, :pw], in_=var[:, :pw], func=FT.Ln,
                    bias=eps_sb[:],
                )
                rstd = smp.tile([1, NCHUNK], f32, tag="rstd")
                nc.scalar.activation(
                    out=rstd[:, :pw], in_=lv[:, :pw], func=FT.Exp,
                    scale=-0.5,
                )
                tsh = smp.tile([1, NCHUNK], f32, tag="tsh")
                nc.vector.tensor_tensor(
                    out=tsh[:, :pw], in0=m1n[:, :pw], in1=rstd[:, :pw], op=OP.mult
                )
                r_bc = bcp.tile([128, NCHUNK], f32, tag="rbc")
                nc.gpsimd.partition_broadcast(out_ap=r_bc[:, :pw], in_ap=rstd[:, :pw])
                t_bc = bcp.tile([128, NCHUNK], f32, tag="tbc")
                nc.gpsimd.partition_broadcast(out_ap=t_bc[:, :pw], in_ap=tsh[:, :pw])
                for cc in range(4):
                    nc.vector.tensor_tensor(
                        out=ctxn[:, cc, p0:p1], in0=ctx_raw[:, cc, p0:p1],
                        in1=r_bc[:, :pw], op=OP.mult,
                    )
                    nc.vector.tensor_tensor(
                        out=ctxn[:, cc, p0:p1], in0=ctxn[:, cc, p0:p1],
                        in1=t_bc[:, :pw], op=OP.add,
                    )
                # K^T (feature-major) for this piece
                for kc in range(4):
                    ps = qpp.tile([128, NCHUNK], f32, tag="q")
                    for cc in range(4):
                        nc.tensor.matmul(
                            ps[:, :pw],
                            lhsT=r(wk_sb[:, cc, kc * 128:(kc + 1) * 128]),
                            rhs=r(ctxn[:, cc, p0:p1]),
                            start=(cc == 0), stop=(cc == 3),
                        )
                    nc.scalar.activation(
                        out=k_sb[:, kc, p0:p1], in_=ps[:, :pw],
                        func=FT.Identity, bias=bk_sb[:, kc:kc + 1],
                    )
                    # per-kc swap copies: k_sw available as soon as each
                    # kc lands, so head-0 scores start earlier
                    nc.vector.tensor_copy(
                        out=k_sw[0:64, kc, p0:p1], in_=k_sb[64:128, kc, p0:p1])
                    nc.vector.tensor_copy(
                        out=k_sw[64:128, kc, p0:p1], in_=k_sb[0:64, kc, p0:p1])
                # V'' (token-major) for this piece's m-chunks
                for mc in range(p0 // 128, p1 // 128):
                    ps = fpp.tile([128, NCHUNK], f32, tag="fin")
                    for cc in range(4):
                        nc.tensor.matmul(
                            ps[:],
                            lhsT=r(ctxn[:, cc, mc * 128:(mc + 1) * 128]),
                            rhs=r(wv_sb[:, cc, :]),
                            start=(cc == 0), stop=(cc == 3),
                        )
                    v3 = v2_sb[:, mc, :].rearrange("p (h e) -> p h e", e=EH)
                    nc.vector.tensor_tensor(
                        out=v3[:, :, 0:DH],
                        in0=ps[:].rearrange("p (h d) -> p h d", d=DH),
                        in1=bv3, op=OP.add,
                    )
                    nc.vector.tensor_scalar(
                        out=v3[:, :, 0:DH], in0=v3[:, :, 0:DH],
                        scalar1=maskc_sb[:, mc:mc + 1], scalar2=None,
                        op0=OP.mult,
                    )
                    nc.vector.tensor_scalar(
                        out=v3[:, :, DH:EH],
                        in0=ones8_sb[:].rearrange("p (h u) -> p h u", u=1),
                        scalar1=maskc_sb[:, mc:mc + 1], scalar2=None,
                        op0=OP.mult,
                    )

            # ---- phase 2: rolled main loop over (chunk, head) ----
            def scores_exp(h, q_sb, q_sw):
                sA = spool.tile([128, 4, NCHUNK], f32, tag="sA", name="sA")
                sB = None
                if nB:
                    sB = spool.tile([128, 1, NCHUNK], f32, tag="sB", name="sB")
                # mc even -> array rows 0-63, mc odd -> rows 64-127, so
                # consecutive m-chunks run concurrently in the PE array.
                for mc in range(nmc):
                    half = mc % 2
                    if (h % 2) == half:
                        ksrc, qsrc = k_sb, q_sb
                    else:
                        ksrc, qsrc = k_sw, q_sw
                    hp = half * 64
                    hc = h // 2
                    dst = sA[:, mc, :] if mc < nA else sB[:, mc - nA, :]
                    nc.tensor.matmul(
                        dst,
                        lhsT=r(ksrc[hp:hp + 64, hc, mc * 128:(mc + 1) * 128]),
                        rhs=r(qsrc[hp:hp + 64, hc, :]),
                        start=True, stop=True,
                    )
                eA = ep.tile([128, nA, NCHUNK], bf16, tag="eA")
                nc.scalar.activation(
                    out=eA[:], in_=sA[:, :nA, :], func=FT.Exp)
                eB = None
                if nB:
                    eB = ep.tile([128, nB, NCHUNK], bf16, tag="eB")
                    nc.scalar.activation(
                        out=eB[:], in_=sB[:, :nB, :], func=FT.Exp)
                return eA, eB

            def pv_drain(h, eA, eB, o_st):
                pv = pvp.tile([EH, NCHUNK], f32, tag="pv")
                for mc in range(nmc):
                    src = eA[:, mc, :] if mc < nA else eB[:, mc - nA, :]
                    nc.tensor.matmul(
                        pv[:],
                        lhsT=r(v2_sb[:, mc, h * EH:(h + 1) * EH]),
                        rhs=r(src),
                        start=(mc == 0), stop=(mc == nmc - 1),
                    )
                nc.vector.tensor_copy(out=o_st[:, h, :], in_=pv[:])

            def half_divide_ll(g4, o_st, o_sb):
                """Tail variant: 1/den via exp(-ln(den)) on the (idle) ACT
                engine -- lower latency than the gpsimd DMA round-trip."""
                ld = r1p.tile([1, 4, NCHUNK], f32, tag="ld")
                nc.scalar.activation(
                    out=ld[:], in_=o_st[DH:EH, 4 * g4:4 * g4 + 4, :],
                    func=FT.Ln)
                rfl = r1p.tile([1, 4, NCHUNK], bf16, tag="rfe")
                nc.scalar.activation(
                    out=rfl[:], in_=ld[:], func=FT.Exp, scale=-1.0)
                for hh in range(4 * g4, 4 * g4 + 4):
                    rb = rbp.tile([64, NCHUNK], bf16, tag="rb")
                    nc.gpsimd.partition_broadcast(
                        out_ap=rb[:], in_ap=rfl[0:1, hh - 4 * g4, :])
                    nc.vector.tensor_tensor(
                        out=o_sb[(hh % 2) * 64:(hh % 2) * 64 + 64,
                                 hh // 2, :],
                        in0=o_st[0:DH, hh, :], in1=rb[:],
                        op=OP.mult,
                    )

            def half_divide(g4, o_st, o_sb):
                dT = r1p.tile([128, 4 * NCHUNK // 128], bf16, tag=f"dT{g4}")
                nc.gpsimd.dma_start(
                    out=dT[:],
                    in_=o_st[DH:EH, 4 * g4:4 * g4 + 4, :].rearrange(
                        "p a b -> p (a b)"))
                rT = r1p.tile([128, 4 * NCHUNK // 128], bf16, tag=f"rT{g4}")
                with nc.allow_low_precision("bf16 softmax denoms"):
                    nc.vector.reciprocal(out=rT[:], in_=dT[:])
                rfl = r1p.tile([1, 4, NCHUNK], bf16, tag=f"rf{g4}")
                nc.gpsimd.dma_start(
                    out=rfl[:].rearrange("p a b -> p (a b)"),
                    in_=rT[:])
                for hh in range(4 * g4, 4 * g4 + 4):
                    rb = rbp.tile([64, NCHUNK], bf16, tag="rb")
                    nc.gpsimd.partition_broadcast(
                        out_ap=rb[:], in_ap=rfl[0:1, hh - 4 * g4, :])
                    nc.vector.tensor_tensor(
                        out=o_sb[(hh % 2) * 64:(hh % 2) * 64 + 64,
                                 hh // 2, :],
                        in0=o_st[0:DH, hh, :], in1=rb[:],
                        op=OP.mult,
                    )

            def emit_proj(c, o_sb):
                n0 = c * NCHUNK
                for cc in range(4):
                    ps = fpp.tile([128, NCHUNK], f32, tag="fin")
                    for ic in range(4):
                        nc.tensor.matmul(
                            ps[:],
                            lhsT=r(wp_sb[:, ic, cc * 128:(cc + 1) * 128]),
                            rhs=r(o_sb[:, ic, :]),
                            start=(ic == 0), stop=(ic == 3),
                        )
                    y_sb = yp.tile([128, NCHUNK], f32, tag="y")
                    nc.vector.tensor_scalar(
                        out=y_sb[:], in0=ps[:], scalar1=bp_sb[:, cc:cc + 1],
                        scalar2=None, op0=OP.add,
                    )
                    nc.sync.dma_start(
                        out=out_e.ap()[cc * 128:(cc + 1) * 128, n0:n0 + NCHUNK],
                        in_=y_sb[:],
                    )

            q_next = None
            o_sb_by_c = {}
            o_st_by_c = {}
            prev = None  # (c, h, eA, eB)
            for g in range(n_nc * HEADS):
                c, h = divmod(g, HEADS)
                if h == 0:
                    o_sb_by_c[c] = opool.tile(
                        [128, 4, NCHUNK], bf16, tag="o", name="o_sb")
                    o_st_by_c[c] = opool.tile(
                        [EH, 8, NCHUNK], bf16, tag="ost", name="o_st")
                e_pair = scores_exp(h, *q_cur)
                if prev is not None:
                    pc, ph, peA, peB = prev
                    pv_drain(ph, peA, peB, o_st_by_c[pc])
                    if ph == 3:
                        half_divide(0, o_st_by_c[pc], o_sb_by_c[pc])
                    elif ph == 7:
                        half_divide(1, o_st_by_c[pc], o_sb_by_c[pc])
                prev = (c, h, *e_pair)
                if h == 2 and c + 1 < n_nc:
                    q_next = emit_q(c + 1)
                elif h == 4 and c > 0:
                    emit_proj(c - 1, o_sb_by_c.pop(c - 1))
                    o_st_by_c.pop(c - 1)
                elif h == 7:
                    q_cur = q_next
            pc, ph, peA, peB = prev
            pv_drain(ph, peA, peB, o_st_by_c[pc])
            half_divide_ll(1, o_st_by_c[pc], o_sb_by_c[pc])
            emit_proj(n_nc - 1, o_sb_by_c.pop(n_nc - 1))

    nc.finalize()
    return nc


def _prep_inputs(x, mask, Wq, Wkv, conv_w, conv_b, ln_w, ln_b, Wp, bp, W):
    """Host-side sharding + layout prep. Returns (in_maps, m_pad)."""
    import ml_dtypes
    bf16 = ml_dtypes.bfloat16
    x = np.ascontiguousarray(np.asarray(x, dtype=np.float32))
    mask = np.asarray(mask, dtype=np.float32)
    Wq = np.asarray(Wq, dtype=np.float32)
    Wkv = np.asarray(Wkv, dtype=np.float32)
    conv_w = np.asarray(conv_w, dtype=np.float32)
    conv_b = np.asarray(conv_b, dtype=np.float32)
    ln_w = np.asarray(ln_w, dtype=np.float32)
    ln_b = np.asarray(ln_b, dtype=np.float32)
    Wp = np.asarray(Wp, dtype=np.float32)
    bp = np.asarray(bp, dtype=np.float32)

    Wm = W // SR
    kb = [int((mask[b] != 0).sum()) for b in range(B)]
    m_pad = max(512, ((max(kb) + 127) // 128) * 128)

    def rearr_w(w):  # [512, 512] -> [128, 4, 512] with [p, cc, :] = w[cc*128+p]
        return np.ascontiguousarray(w.reshape(4, 128, -1).transpose(1, 0, 2))

    def rearr_b(v):  # [512] -> [128, 4]
        return np.ascontiguousarray(v.reshape(4, 128).T)

    w2 = np.ascontiguousarray(
        conv_w.transpose(2, 3, 1, 0).reshape(4 * DIM, DIM)).astype(bf16)
    wq_in = rearr_w(Wq.T * np.float32(SCALE)).astype(bf16)
    wk_in = rearr_w((Wkv[:INNER] * ln_w).T).astype(bf16)
    wv_in = rearr_w((Wkv[INNER:] * ln_w).T).astype(bf16)
    wp_in = rearr_w(Wp.T).astype(bf16)
    bk_in = rearr_b(Wkv[:INNER] @ ln_b)
    bv_in = np.ascontiguousarray(Wkv[INNER:] @ ln_b)
    convb_in = rearr_b(conv_b)
    bp_in = rearr_b(bp)

    in_maps = []
    for b in range(B):
        xb = x[b]
        sel = np.nonzero(mask[b] != 0)[0]
        sel_pad = np.zeros(m_pad, dtype=np.int64)
        sel_pad[: len(sel)] = sel
        i = sel_pad // Wm
        j = sel_pad % Wm
        n_idx = np.stack(
            [(2 * i + di) * W + (2 * j + dj) for di in (0, 1) for dj in (0, 1)]
        )  # [4, m_pad], p = di*2+dj
        xp = xb[n_idx]  # [4, m_pad, 512]
        xp = np.ascontiguousarray(
            xp.transpose(0, 2, 1).reshape(4 * DIM, m_pad))
        maskc = (np.arange(m_pad) < len(sel)).astype(np.float32)
        maskc_in = np.ascontiguousarray(maskc.reshape(-1, 128).T)
        in_maps.append({
            "xt": np.ascontiguousarray(xb.T).astype(bf16),
            "xp": xp.astype(bf16),
            "w2": w2,
            "wq": wq_in,
            "wk": wk_in,
            "wv": wv_in,
            "wp": wp_in,
            "convb": convb_in,
            "bk": bk_in,
            "bv": bv_in,
            "bp": bp_in,
            "maskc": maskc_in,
        })
    return in_maps, m_pad


_BUILD_CACHE = {}


def kernel(x, H, W, mask, Wq, Wkv, conv_w, conv_b, ln_w, ln_b, Wp, bp,
           _results_hook=None):
    H = int(H)
    W = int(W)
    assert (H, W) == (64, 64) and x.shape == (B, N_SEQ, DIM), (H, W, x.shape)

    in_maps, m_pad = _prep_inputs(
        x, mask, Wq, Wkv, conv_w, conv_b, ln_w, ln_b, Wp, bp, W)

    if m_pad not in _BUILD_CACHE:
        _BUILD_CACHE[m_pad] = _build(m_pad)
    nc = _BUILD_CACHE[m_pad]

    _ensure_path()
    from concourse.bass_utils import run_bass_kernel_spmd

    res = run_bass_kernel_spmd(nc, in_maps, core_ids=list(range(B)))
    if _results_hook is not None:
        _results_hook(res)

    out = np.empty((B, N_SEQ, DIM), dtype=np.float32)
    for b in range(B):
        out[b] = res.results[b]["out"].T
    return out


# revision 21
# speedup vs baseline: 1.2636x; 1.0116x over previous
"""Spatial-reduction attention (PVT-style) on 8 TRN2 NeuronCores.

Strategy: pure data-parallel over batch (B=8 -> 1 batch element per core,
zero collectives). Per core, everything is computed in "feature-major"
(transposed) layout so that the attention-weight matrix E^T = exp(S^T)
lands with the context dim m on partitions -- exactly what the PV matmul
needs as its moving operand, so the big attention tensor is never
transposed on chip.

Key tricks (v3):
  - conv(stride 2, 2x2) == patch-merge matmul; patches are gathered
    host-side, only for the m positions with mask!=0 (mask compression,
    1024 -> m_pad 640), since masked context positions contribute
    nothing to the attention output.
  - mask + softmax denominator are folded into the PV matmul: the
    stationary operand V'' has 65 columns per head (64 = mask*V, 1 = mask),
    so row 64 of the PV output is the softmax denominator.
  - layernorm's ln_w/ln_b are folded into Wkv host-side; on-chip LN is a
    pure standardize using ones-matmul column stats + partition broadcast.
  - all matmuls run in bf16 (fp8 measured 8e-2 rel err -- the output has
    ~1/sqrt(Neff) signal shrinkage, so per-element noise is amplified
    ~12x; bf16 is the cheapest legal dtype).
  - the head loop is software-pipelined ACROSS n-chunks: one rolled
    64-iteration (chunk, head) loop; Q(c+1) is emitted at (c, h==2) and
    proj(c-1) at (c, h==4), so neither the PE nor the ACT exp stream
    stalls at chunk boundaries.
  - m pieces are (512, 128): the first 4 m-chunks (sA/eA, 4 PSUM banks)
    and the 5th (sB/eB, 1 bank) pipeline independently, and the piece-0
    prologue covers 4/5 of the context so attention can start early.
  - output is produced transposed ([512, 4096] per core) and untransposed
    on the host.
"""

import math
import numpy as np

N_SEQ = 4096
DIM = 512
HEADS = 8
DH = 64
INNER = HEADS * DH
SR = 2
SCALE = DH ** -0.5
LN_EPS = 1e-5
B = 8
NCHUNK = 512          # n-tile size of the main loop
EH = DH + 1           # 65: V'' columns per head (64 V + 1 mask/denominator)


def _ensure_path():
    try:
        import concourse.bass  # noqa: F401
    except ImportError:
        import sys
        for p in ("/opt/trn_rl_repo", "/root/.axon_site/_ro/trn_rl_repo"):
            if p not in sys.path:
                sys.path.append(p)


def _build(m_pad):
    _ensure_path()
    import concourse.bass as bass  # noqa: F401
    import concourse.mybir as mybir
    import concourse.tile as tile
    from concourse import bacc

    f32 = mybir.dt.float32
    bf16 = mybir.dt.bfloat16
    FT = mybir.ActivationFunctionType
    OP = mybir.AluOpType

    nmc = m_pad // 128
    assert nmc in (4, 5), nmc
    nA = min(4, nmc)
    nB = nmc - nA
    pieces = [(0, min(512, m_pad))]
    if m_pad > 512:
        pieces.append((512, m_pad))
    n_nc = N_SEQ // NCHUNK

    nc = bacc.Bacc()

    xt_e = nc.declare_dram_parameter("xt", [DIM, N_SEQ], bf16, isOutput=False)
    xp_e = nc.declare_dram_parameter("xp", [4 * DIM, m_pad], bf16, isOutput=False)
    w2_e = nc.declare_dram_parameter("w2", [4 * DIM, DIM], bf16, isOutput=False)
    wq_e = nc.declare_dram_parameter("wq", [128, 4, DIM], bf16, isOutput=False)
    wk_e = nc.declare_dram_parameter("wk", [128, 4, DIM], bf16, isOutput=False)
    wv_e = nc.declare_dram_parameter("wv", [128, 4, DIM], bf16, isOutput=False)
    wp_e = nc.declare_dram_parameter("wp", [128, 4, DIM], bf16, isOutput=False)
    convb_e = nc.declare_dram_parameter("convb", [128, 4], f32, isOutput=False)
    bk_e = nc.declare_dram_parameter("bk", [128, 4], f32, isOutput=False)
    bv_e = nc.declare_dram_parameter("bv", [DIM], f32, isOutput=False)
    bp_e = nc.declare_dram_parameter("bp", [128, 4], f32, isOutput=False)
    maskc_e = nc.declare_dram_parameter("maskc", [128, nmc], f32, isOutput=False)
    out_e = nc.declare_dram_parameter("out", [DIM, N_SEQ], f32, isOutput=True)

    def r(ap):
        return ap

    from contextlib import ExitStack

    with tile.TileContext(nc) as tc:
        with ExitStack() as stk:
            def pool(name, bufs, space="SBUF"):
                return stk.enter_context(
                    tc.tile_pool(name=name, bufs=bufs, space=space))

            wpool = pool("wts", 1)
            cpool = pool("consts", 1)
            xpcp = pool("stream", 1)
            w2p = pool("w2s", 1)
            ctxp = pool("ctx", 1)
            sqp = pool("sqs", 2)
            kvp = pool("kv", 1)
            xtp = pool("xtq", 2)
            qp = pool("qq", 2)
            ep = pool("ee", 2)
            opool = pool("oo", 2)
            yp = pool("yy", 3)
            smp = pool("small", 1)
            r1p = pool("r1p", 2)
            bcp = pool("bc", 1)
            rbp = pool("rbp", 2)
            spool = pool("ps_s", 1, space="PSUM")
            pvp = pool("ps_pv", 1, space="PSUM")
            qpp = pool("ps_q", 1, space="PSUM")
            fpp = pool("ps_f", 1, space="PSUM")

            # ---- constants / weights (resident) ----
            wq_sb = wpool.tile([128, 4, DIM], bf16, tag="wq")
            nc.gpsimd.dma_start(out=wq_sb[:], in_=wq_e.ap())
            wk_sb = wpool.tile([128, 4, DIM], bf16, tag="wk")
            nc.gpsimd.dma_start(out=wk_sb[:], in_=wk_e.ap())
            wv_sb = wpool.tile([128, 4, DIM], bf16, tag="wv")
            nc.gpsimd.dma_start(out=wv_sb[:], in_=wv_e.ap())
            wp_sb = wpool.tile([128, 4, DIM], bf16, tag="wp")
            nc.gpsimd.dma_start(out=wp_sb[:], in_=wp_e.ap())
            convb_sb = cpool.tile([128, 4], f32, tag="convb")
            nc.gpsimd.dma_start(out=convb_sb[:], in_=convb_e.ap())
            bk_sb = cpool.tile([128, 4], f32, tag="bk")
            nc.gpsimd.dma_start(out=bk_sb[:], in_=bk_e.ap())
            bp_sb = cpool.tile([128, 4], f32, tag="bp")
            nc.gpsimd.dma_start(out=bp_sb[:], in_=bp_e.ap())
            maskc_sb = cpool.tile([128, nmc], f32, tag="maskc")
            nc.gpsimd.dma_start(out=maskc_sb[:], in_=maskc_e.ap())
            bv_bc = cpool.tile([128, DIM], f32, tag="bvbc")
            nc.gpsimd.dma_start(out=bv_bc[:], in_=bv_e.ap().partition_broadcast(128))
            ones_sb = cpool.tile([128, 1], bf16, tag="ones")
            nc.vector.memset(ones_sb[:], 1.0)
            ones8_sb = cpool.tile([128, 8], f32, tag="ones8")
            nc.vector.memset(ones8_sb[:], 1.0)
            eps_sb = cpool.tile([1, 1], f32, tag="eps")
            nc.vector.memset(eps_sb[:], LN_EPS)

            # xt chunk-0 prefetch, issued before the bulk xp/w2 stream
            xt_r = xt_e.ap().rearrange("(cc p) n -> p cc n", p=128)
            xt_pre = xtp.tile([128, 4, NCHUNK], bf16, tag="xt")
            nc.sync.dma_start(out=xt_pre[:], in_=xt_r[:, :, 0:NCHUNK])

            xp_r = xp_e.ap().rearrange("(kc p) m -> kc p m", p=128)
            w2_r = w2_e.ap().rearrange("(kc p) co -> kc p co", p=128)
            xp_t = []
            w2_t = []
            for kc in range(16):
                xpk = xpcp.tile([128, m_pad], bf16, tag=f"xp{kc}")
                nc.sync.dma_start(out=xpk[:], in_=xp_r[kc, :, :])
                xp_t.append(xpk)
                w2k = w2p.tile([128, DIM], bf16, tag=f"w2{kc}")
                nc.sync.dma_start(out=w2k[:], in_=w2_r[kc, :, :])
                w2_t.append(w2k)

            # ---- Q projection (chunk c) ----
            def emit_q(c):
                n0 = c * NCHUNK
                if c == 0:
                    xt_sb = xt_pre
                else:
                    xt_sb = xtp.tile([128, 4, NCHUNK], bf16, tag="xt")
                    nc.sync.dma_start(
                        out=xt_sb[:], in_=xt_r[:, :, n0:n0 + NCHUNK])
                q_sb = qp.tile([128, 4, NCHUNK], bf16, tag="q")
                for ic in range(4):
                    ps = qpp.tile([128, NCHUNK], f32, tag="q")
                    for cc in range(4):
                        nc.tensor.matmul(
                            ps[:],
                            lhsT=r(wq_sb[:, cc, ic * 128:(ic + 1) * 128]),
                            rhs=r(xt_sb[:, cc, :]),
                            start=(cc == 0), stop=(cc == 3),
                        )
                    nc.vector.tensor_copy(out=q_sb[:, ic, :], in_=ps[:])
                q_sw = qp.tile([128, 4, NCHUNK], bf16, tag="qsw")
                nc.vector.tensor_copy(out=q_sw[0:64, :, :], in_=q_sb[64:128, :, :])
                nc.vector.tensor_copy(out=q_sw[64:128, :, :], in_=q_sb[0:64, :, :])
                return q_sb, q_sw

            # Q(0) early: only needs xt chunk 0, warms the PE while the
            # patch stream is still loading.
            q_cur = emit_q(0)

            # ---- phase 1: patch-merge ctx^T + layernorm + K/V, per piece ----
            ctx_raw = ctxp.tile([128, 4, m_pad], bf16, tag="craw")
            ctxn = ctxp.tile([128, 4, m_pad], bf16, tag="cn")
            k_sb = kvp.tile([128, 4, m_pad], bf16, tag="k")
            k_sw = kvp.tile([128, 4, m_pad], bf16, tag="ksw")
            v2_sb = kvp.tile([128, nmc, HEADS * EH], bf16, tag="v2")
            bv3 = bv_bc[:].rearrange("p (h d) -> p h d", d=DH)

            def emit_patch(p0, p1):
                pw = p1 - p0
                psA = spool.tile([128, 4, NCHUNK], f32, tag="sA", name="psA")
                for kc in range(16):
                    for cco in range(4):
                        nc.tensor.matmul(
                            psA[:, cco, :pw],
                            lhsT=r(w2_t[kc][:, cco * 128:(cco + 1) * 128]),
                            rhs=r(xp_t[kc][:, p0:p1]),
                            start=(kc == 0), stop=(kc == 15),
                        )
                for cco in range(4):
                    nc.scalar.activation(
                        out=ctx_raw[:, cco, p0:p1], in_=psA[:, cco, :pw],
                        func=FT.Identity, bias=convb_sb[:, cco:cco + 1],
                    )

            def emit_ln(p0, p1):
                pw = p1 - p0
                # column stats via ones-matmul (sum over the c partition dim)
                mu_ps = pvp.tile([EH, NCHUNK], f32, tag="pv", name="mu_ps")
                ss_ps = fpp.tile([128, NCHUNK], f32, tag="fin", name="ss_ps")
                for cc in range(4):
                    sq_s = sqp.tile([128, NCHUNK], bf16, tag="sqs")
                    nc.vector.tensor_tensor(
                        out=sq_s[:, :pw], in0=ctx_raw[:, cc, p0:p1],
                        in1=ctx_raw[:, cc, p0:p1], op=OP.mult,
                    )
                    nc.tensor.matmul(
                        mu_ps[0:1, :pw], lhsT=r(ones_sb[:]),
                        rhs=r(ctx_raw[:, cc, p0:p1]),
                        start=(cc == 0), stop=(cc == 3),
                    )
                    nc.tensor.matmul(
                        ss_ps[0:1, :pw], lhsT=r(ones_sb[:]),
                        rhs=r(sq_s[:, :pw]),
                        start=(cc == 0), stop=(cc == 3),
                    )
                m1n = smp.tile([1, NCHUNK], f32, tag="m1n")
                nc.vector.tensor_scalar(
                    out=m1n[:, :pw], in0=mu_ps[0:1, :pw],
                    scalar1=-1.0 / DIM, scalar2=None, op0=OP.mult,
                )
                v1 = smp.tile([1, NCHUNK], f32, tag="v1")
                nc.vector.tensor_scalar(
                    out=v1[:, :pw], in0=ss_ps[0:1, :pw],
                    scalar1=1.0 / DIM, scalar2=None, op0=OP.mult,
                )
                m2 = smp.tile([1, NCHUNK], f32, tag="m2")
                nc.vector.tensor_tensor(
                    out=m2[:, :pw], in0=m1n[:, :pw], in1=m1n[:, :pw], op=OP.mult
                )
                var = smp.tile([1, NCHUNK], f32, tag="var")
                nc.vector.tensor_tensor(
                    out=var[:, :pw], in0=v1[:, :pw], in1=m2[:, :pw], op=OP.subtract
                )
                std = smp.tile([1, NCHUNK], f32, tag="std")
                nc.scalar.activation(
                    out=std[:, :pw], in_=var[:, :pw], func=FT.Sqrt,
                    bias=eps_sb[:],
                )
                rstd = smp.tile([1, NCHUNK], f32, tag="rstd")
                nc.vector.reciprocal(out=rstd[:, :pw], in_=std[:, :pw])
                tsh = smp.tile([1, NCHUNK], f32, tag="tsh")
                nc.vector.tensor_tensor(
                    out=tsh[:, :pw], in0=m1n[:, :pw], in1=rstd[:, :pw], op=OP.mult
                )
                r_bc = bcp.tile([128, NCHUNK], f32, tag="rbc")
                nc.gpsimd.partition_broadcast(out_ap=r_bc[:, :pw], in_ap=rstd[:, :pw])
                t_bc = bcp.tile([128, NCHUNK], f32, tag="tbc")
                nc.gpsimd.partition_broadcast(out_ap=t_bc[:, :pw], in_ap=tsh[:, :pw])
                for cc in range(4):
                    nc.vector.tensor_tensor(
                        out=ctxn[:, cc, p0:p1], in0=ctx_raw[:, cc, p0:p1],
                        in1=r_bc[:, :pw], op=OP.mult,
                    )
                    nc.vector.tensor_tensor(
                        out=ctxn[:, cc, p0:p1], in0=ctxn[:, cc, p0:p1],
                        in1=t_bc[:, :pw], op=OP.add,
                    )

            def emit_k(p0, p1):
                pw = p1 - p0
                for kc in range(4):
                    ps = qpp.tile([128, NCHUNK], f32, tag="q", name="kps")
                    for cc in range(4):
                        nc.tensor.matmul(
                            ps[:, :pw],
                            lhsT=r(wk_sb[:, cc, kc * 128:(kc + 1) * 128]),
                            rhs=r(ctxn[:, cc, p0:p1]),
                            start=(cc == 0), stop=(cc == 3),
                        )
                    nc.scalar.activation(
                        out=k_sb[:, kc, p0:p1], in_=ps[:, :pw],
                        func=FT.Identity, bias=bk_sb[:, kc:kc + 1],
                    )
                    # per-kc swap copies: k_sw available as soon as each
                    # kc lands, so head-0 scores start earlier
                    nc.vector.tensor_copy(
                        out=k_sw[0:64, kc, p0:p1], in_=k_sb[64:128, kc, p0:p1])
                    nc.vector.tensor_copy(
                        out=k_sw[64:128, kc, p0:p1], in_=k_sb[0:64, kc, p0:p1])

            def emit_v(mc):
                ps = fpp.tile([128, NCHUNK], f32, tag="fin", name="vps")
                for cc in range(4):
                    nc.tensor.matmul(
                        ps[:],
                        lhsT=r(ctxn[:, cc, mc * 128:(mc + 1) * 128]),
                        rhs=r(wv_sb[:, cc, :]),
                        start=(cc == 0), stop=(cc == 3),
                    )
                v3 = v2_sb[:, mc, :].rearrange("p (h e) -> p h e", e=EH)
                nc.vector.tensor_tensor(
                    out=v3[:, :, 0:DH],
                    in0=ps[:].rearrange("p (h d) -> p h d", d=DH),
                    in1=bv3, op=OP.add,
                )
                nc.vector.tensor_scalar(
                    out=v3[:, :, 0:DH], in0=v3[:, :, 0:DH],
                    scalar1=maskc_sb[:, mc:mc + 1], scalar2=None,
                    op0=OP.mult,
                )
                nc.vector.tensor_scalar(
                    out=v3[:, :, DH:EH],
                    in0=ones8_sb[:].rearrange("p (h u) -> p h u", u=1),
                    scalar1=maskc_sb[:, mc:mc + 1], scalar2=None,
                    op0=OP.mult,
                )

            # patch both pieces back-to-back on the PE (piece-1 reuses the
            # sA banks as soon as piece-0's Identity drains them), then
            # LN -> K with K for all pieces before any V: the critical path
            # to the first exp runs through K only.
            for (p0, p1) in pieces:
                emit_patch(p0, p1)
            for (p0, p1) in pieces:
                emit_ln(p0, p1)
            for (p0, p1) in pieces:
                emit_k(p0, p1)
            for mc in range(nmc):
                emit_v(mc)

            # ---- phase 2: rolled main loop over (chunk, head) ----
            def scores_exp(h, q_sb, q_sw):
                sA = spool.tile([128, 4, NCHUNK], f32, tag="sA", name="sA")
                sB = None
                if nB:
                    sB = spool.tile([128, 1, NCHUNK], f32, tag="sB", name="sB")
                # mc even -> array rows 0-63, mc odd -> rows 64-127, so
                # consecutive m-chunks run concurrently in the PE array.
                for mc in range(nmc):
                    half = mc % 2
                    if (h % 2) == half:
                        ksrc, qsrc = k_sb, q_sb
                    else:
                        ksrc, qsrc = k_sw, q_sw
                    hp = half * 64
                    hc = h // 2
                    dst = sA[:, mc, :] if mc < nA else sB[:, mc - nA, :]
                    nc.tensor.matmul(
                        dst,
                        lhsT=r(ksrc[hp:hp + 64, hc, mc * 128:(mc + 1) * 128]),
                        rhs=r(qsrc[hp:hp + 64, hc, :]),
                        start=True, stop=True,
                    )
                eA = ep.tile([128, nA, NCHUNK], bf16, tag="eA")
                nc.scalar.activation(
                    out=eA[:], in_=sA[:, :nA, :], func=FT.Exp)
                eB = None
                if nB:
                    eB = ep.tile([128, nB, NCHUNK], bf16, tag="eB")
                    nc.scalar.activation(
                        out=eB[:], in_=sB[:, :nB, :], func=FT.Exp)
                return eA, eB

            def pv_drain(h, eA, eB, o_st):
                pv = pvp.tile([EH, NCHUNK], f32, tag="pv")
                for mc in range(nmc):
                    src = eA[:, mc, :] if mc < nA else eB[:, mc - nA, :]
                    nc.tensor.matmul(
                        pv[:],
                        lhsT=r(v2_sb[:, mc, h * EH:(h + 1) * EH]),
                        rhs=r(src),
                        start=(mc == 0), stop=(mc == nmc - 1),
                    )
                nc.vector.tensor_copy(out=o_st[:, h, :], in_=pv[:])

            def half_divide_ll(g4, o_st, o_sb):
                """Tail variant: 1/den via exp(-ln(den)) on the (idle) ACT
                engine -- lower latency than the gpsimd DMA round-trip."""
                ld = r1p.tile([1, 4, NCHUNK], f32, tag="ld")
                nc.scalar.activation(
                    out=ld[:], in_=o_st[DH:EH, 4 * g4:4 * g4 + 4, :],
                    func=FT.Ln)
                rfl = r1p.tile([1, 4, NCHUNK], bf16, tag="rfe")
                nc.scalar.activation(
                    out=rfl[:], in_=ld[:], func=FT.Exp, scale=-1.0)
                for hh in range(4 * g4, 4 * g4 + 4):
                    rb = rbp.tile([64, NCHUNK], bf16, tag="rb")
                    nc.gpsimd.partition_broadcast(
                        out_ap=rb[:], in_ap=rfl[0:1, hh - 4 * g4, :])
                    nc.vector.tensor_tensor(
                        out=o_sb[(hh % 2) * 64:(hh % 2) * 64 + 64,
                                 hh // 2, :],
                        in0=o_st[0:DH, hh, :], in1=rb[:],
                        op=OP.mult,
                    )

            def half_divide(g4, o_st, o_sb):
                dT = r1p.tile([128, 4 * NCHUNK // 128], bf16, tag=f"dT{g4}")
                nc.gpsimd.dma_start(
                    out=dT[:],
                    in_=o_st[DH:EH, 4 * g4:4 * g4 + 4, :].rearrange(
                        "p a b -> p (a b)"))
                rT = r1p.tile([128, 4 * NCHUNK // 128], bf16, tag=f"rT{g4}")
                with nc.allow_low_precision("bf16 softmax denoms"):
                    nc.vector.reciprocal(out=rT[:], in_=dT[:])
                rfl = r1p.tile([1, 4, NCHUNK], bf16, tag=f"rf{g4}")
                nc.gpsimd.dma_start(
                    out=rfl[:].rearrange("p a b -> p (a b)"),
                    in_=rT[:])
                for hh in range(4 * g4, 4 * g4 + 4):
                    rb = rbp.tile([64, NCHUNK], bf16, tag="rb")
                    nc.gpsimd.partition_broadcast(
                        out_ap=rb[:], in_ap=rfl[0:1, hh - 4 * g4, :])
                    nc.vector.tensor_tensor(
                        out=o_sb[(hh % 2) * 64:(hh % 2) * 64 + 64,
                                 hh // 2, :],
                        in0=o_st[0:DH, hh, :], in1=rb[:],
                        op=OP.mult,
                    )

            def emit_proj(c, o_sb):
                n0 = c * NCHUNK
                for cc in range(4):
                    ps = fpp.tile([128, NCHUNK], f32, tag="fin")
                    for ic in range(4):
                        nc.tensor.matmul(
                            ps[:],
                            lhsT=r(wp_sb[:, ic, cc * 128:(cc + 1) * 128]),
                            rhs=r(o_sb[:, ic, :]),
                            start=(ic == 0), stop=(ic == 3),
                        )
                    y_sb = yp.tile([128, NCHUNK], f32, tag="y")
                    nc.vector.tensor_scalar(
                        out=y_sb[:], in0=ps[:], scalar1=bp_sb[:, cc:cc + 1],
                        scalar2=None, op0=OP.add,
                    )
                    nc.sync.dma_start(
                        out=out_e.ap()[cc * 128:(cc + 1) * 128, n0:n0 + NCHUNK],
                        in_=y_sb[:],
                    )

            q_next = None
            o_sb_by_c = {}
            o_st_by_c = {}
            prev = None  # (c, h, eA, eB)
            for g in range(n_nc * HEADS):
                c, h = divmod(g, HEADS)
                if h == 0:
                    o_sb_by_c[c] = opool.tile(
                        [128, 4, NCHUNK], bf16, tag="o", name="o_sb")
                    o_st_by_c[c] = opool.tile(
                        [EH, 8, NCHUNK], bf16, tag="ost", name="o_st")
                e_pair = scores_exp(h, *q_cur)
                if prev is not None:
                    pc, ph, peA, peB = prev
                    pv_drain(ph, peA, peB, o_st_by_c[pc])
                    if ph == 3:
                        half_divide(0, o_st_by_c[pc], o_sb_by_c[pc])
                    elif ph == 7:
                        half_divide(1, o_st_by_c[pc], o_sb_by_c[pc])
                prev = (c, h, *e_pair)
                if h == 2 and c + 1 < n_nc:
                    q_next = emit_q(c + 1)
                elif h == 4 and c > 0:
                    emit_proj(c - 1, o_sb_by_c.pop(c - 1))
                    o_st_by_c.pop(c - 1)
                elif h == 7:
                    q_cur = q_next
            pc, ph, peA, peB = prev
            pv_drain(ph, peA, peB, o_st_by_c[pc])
            half_divide(1, o_st_by_c[pc], o_sb_by_c[pc])
            emit_proj(n_nc - 1, o_sb_by_c.pop(n_nc - 1))

    nc.finalize()
    return nc


def _prep_inputs(x, mask, Wq, Wkv, conv_w, conv_b, ln_w, ln_b, Wp, bp, W):
    """Host-side sharding + layout prep. Returns (in_maps, m_pad)."""
    import ml_dtypes
    bf16 = ml_dtypes.bfloat16
    x = np.ascontiguousarray(np.asarray(x, dtype=np.float32))
    mask = np.asarray(mask, dtype=np.float32)
    Wq = np.asarray(Wq, dtype=np.float32)
    Wkv = np.asarray(Wkv, dtype=np.float32)
    conv_w = np.asarray(conv_w, dtype=np.float32)
    conv_b = np.asarray(conv_b, dtype=np.float32)
    ln_w = np.asarray(ln_w, dtype=np.float32)
    ln_b = np.asarray(ln_b, dtype=np.float32)
    Wp = np.asarray(Wp, dtype=np.float32)
    bp = np.asarray(bp, dtype=np.float32)

    Wm = W // SR
    kb = [int((mask[b] != 0).sum()) for b in range(B)]
    m_pad = max(512, ((max(kb) + 127) // 128) * 128)

    def rearr_w(w):  # [512, 512] -> [128, 4, 512] with [p, cc, :] = w[cc*128+p]
        return np.ascontiguousarray(w.reshape(4, 128, -1).transpose(1, 0, 2))

    def rearr_b(v):  # [512] -> [128, 4]
        return np.ascontiguousarray(v.reshape(4, 128).T)

    w2 = np.ascontiguousarray(
        conv_w.transpose(2, 3, 1, 0).reshape(4 * DIM, DIM)).astype(bf16)
    wq_in = rearr_w(Wq.T * np.float32(SCALE)).astype(bf16)
    wk_in = rearr_w((Wkv[:INNER] * ln_w).T).astype(bf16)
    wv_in = rearr_w((Wkv[INNER:] * ln_w).T).astype(bf16)
    wp_in = rearr_w(Wp.T).astype(bf16)
    bk_in = rearr_b(Wkv[:INNER] @ ln_b)
    bv_in = np.ascontiguousarray(Wkv[INNER:] @ ln_b)
    convb_in = rearr_b(conv_b)
    bp_in = rearr_b(bp)

    in_maps = []
    for b in range(B):
        xb = x[b]
        sel = np.nonzero(mask[b] != 0)[0]
        sel_pad = np.zeros(m_pad, dtype=np.int64)
        sel_pad[: len(sel)] = sel
        i = sel_pad // Wm
        j = sel_pad % Wm
        n_idx = np.stack(
            [(2 * i + di) * W + (2 * j + dj) for di in (0, 1) for dj in (0, 1)]
        )  # [4, m_pad], p = di*2+dj
        xp = xb[n_idx]  # [4, m_pad, 512]
        xp = np.ascontiguousarray(
            xp.transpose(0, 2, 1).reshape(4 * DIM, m_pad))
        maskc = (np.arange(m_pad) < len(sel)).astype(np.float32)
        maskc_in = np.ascontiguousarray(maskc.reshape(-1, 128).T)
        in_maps.append({
            "xt": np.ascontiguousarray(xb.T).astype(bf16),
            "xp": xp.astype(bf16),
            "w2": w2,
            "wq": wq_in,
            "wk": wk_in,
            "wv": wv_in,
            "wp": wp_in,
            "convb": convb_in,
            "bk": bk_in,
            "bv": bv_in,
            "bp": bp_in,
            "maskc": maskc_in,
        })
    return in_maps, m_pad


_BUILD_CACHE = {}


def kernel(x, H, W, mask, Wq, Wkv, conv_w, conv_b, ln_w, ln_b, Wp, bp,
           _results_hook=None):
    H = int(H)
    W = int(W)
    assert (H, W) == (64, 64) and x.shape == (B, N_SEQ, DIM), (H, W, x.shape)

    in_maps, m_pad = _prep_inputs(
        x, mask, Wq, Wkv, conv_w, conv_b, ln_w, ln_b, Wp, bp, W)

    if m_pad not in _BUILD_CACHE:
        _BUILD_CACHE[m_pad] = _build(m_pad)
    nc = _BUILD_CACHE[m_pad]

    _ensure_path()
    from concourse.bass_utils import run_bass_kernel_spmd

    res = run_bass_kernel_spmd(nc, in_maps, core_ids=list(range(B)))
    if _results_hook is not None:
        _results_hook(res)

    out = np.empty((B, N_SEQ, DIM), dtype=np.float32)
    for b in range(B):
        out[b] = res.results[b]["out"].T
    return out
